# revision 1
# baseline (speedup 1.0000x reference)
"""NeuralCDE forward on 8 Trainium2 NeuronCores.

Strategy: pure data parallelism — 64-batch split as 8 per core. Each core
runs the sequential RK4 scan (127 intervals x 4 substeps x 4 vf evals)
with activations feature-major [feat, batch=8]:

  vf MLP: h0 = softplus(fW0 @ y + fb0); h = softplus(fWh[k] @ h + fbh[k]) x3
          z = fWo @ h3 (+fbo), t = tanh(z), g = reshape(t,(64,33)) @ xdot

- softplus = ln(1 + exp(u)) via ACT Exp then Ln(bias=1) — the only ACT
  table set holding both funcs (natural_log_exp_and_others); native
  Softplus has no table set in this toolchain.
- tanh(v) = 1 - 2/(e^{2v}+1): ACT Exp(scale=2) + DVE divide. The xdot
  contraction is folded: t*x = x - 2*x/(e^{2v}+1); sum over C via a DVE
  reduce + one small S-matmul; RK4 stage/combine scales are pre-folded
  into host-precomputed xdot replicas (xrep) DMA'd per eval.
- fWo rows are permuted to c-major (row' = c*64 + h, c padded 33->34) so
  z chunks align with a per-partition (c,h) layout.
- Host precomputes: Hermite coeffs -> scaled xdot replicas, y0 init MLP,
  final readout ysol @ lW.T + lb.
"""

import numpy as np

N_CORES = 8
T = 128
B = 64
OBS = 32
HID = 64
WID = 128
OUT = 32
C = OBS + 1          # 33
CP = 34              # padded C (even)
NCHUNK = 17          # 2176 / 128
ZF = NCHUNK * 8      # 136 free cols of the z tile
XF = ZF + 8          # 144: xrep ++ xrepsum
NSUB = 4
NI = T - 1           # 127 intervals
BL = B // N_CORES    # 8 per core

_COMPILED = None     # cache across calls
_LAST_IN_MAPS = None  # stashed for test.py profiling

# fp16 MLP weights + activations-as-rhs: halves the PE weight-stream cost
# (fp32 stationaries lower to two LDWEIGHTS+MATMUL passes). PSUM still
# accumulates fp32; RK4 state y stays fp32.
BF16_WEIGHTS = True
# fbo is zeros in this problem's setup_inputs, so exp(2*fbo)=1 and the
# E*Frep multiply is dropped from the tanh head. Checked at runtime.
ASSUME_FBO_ZERO = True
STAGGERED = True


# ----------------------------------------------------------------- host math

def _host_precompute(ts, ys, iW0, ib0, iWh, ibh, iWo, ibo, fWo, fbo):
    """Returns per-core input arrays (all fp32)."""
    f32 = np.float32
    ts = ts.astype(f32)
    ys = ys.astype(f32)

    # control path pieces (all batch at once), mirrors reference `single`
    tys = np.concatenate([np.broadcast_to(ts[None, :, None], (B, T, 1)), ys], axis=-1)
    dts = ts[1:] - ts[:-1]                                  # (NI,)
    diffs = (tys[:, 1:] - tys[:, :-1]) / dts[None, :, None]  # (B, NI, C)
    deriv = np.concatenate([diffs[:, :1], diffs], axis=1)    # (B, T, C)
    d0 = deriv[:, :-1]                                       # (B, NI, C)
    d1 = deriv[:, 1:]                                        # (B, NI, C)
    cc = (3.0 * diffs - 2.0 * d0 - d1) / dts[None, :, None]
    bb = (d0 + d1 - 2.0 * diffs) / (dts * dts)[None, :, None]

    # xdot at the 3 distinct points per substep, with RK4 combine scales
    # folded in: X_p = scale_p * xdot(s_p), scale = hs/6, hs/3, hs/6.
    hs = dts / NSUB                                          # (NI,)
    k_idx = np.arange(NSUB, dtype=f32)                       # (4,)
    # s points: (NI, 4, 3)
    s0 = k_idx[None, :] * hs[:, None]
    s_pts = np.stack([s0, s0 + hs[:, None] / 2, s0 + hs[:, None]], axis=-1)
    w_pts = np.stack([hs / 6, hs / 3, hs / 6], axis=-1)      # (NI, 3)

    # xdot(b, n, k, p, c) = d0 + 2 c s + 3 b s^2
    s = s_pts[None, :, :, :, None]                           # (1, NI, 4, 3, 1)
    xd = (d0[:, :, None, None, :]
          + 2.0 * cc[:, :, None, None, :] * s
          + 3.0 * bb[:, :, None, None, :] * s * s)           # (B, NI, 4, 3, C)
    xd = xd * w_pts[None, :, None, :, None]                  # fold scales
    xdp = np.zeros((B, NI, NSUB, 3, CP), f32)
    xdp[..., :C] = xd

    # xrep[part, 8q+b] = X[b, 2q + part//64]; xrepsum = sum_q xrep
    # build per core to keep memory reasonable
    q_idx = np.arange(NCHUNK)
    part_half = np.arange(128) // 64                          # (128,)
    cmap = (2 * q_idx[None, :] + part_half[:, None])          # (128, 17)

    xr_cores = []
    for core in range(N_CORES):
        sl = slice(core * BL, (core + 1) * BL)
        X = xdp[sl]                                           # (8, NI, 4, 3, CP)
        # xrep: (NI, 4, 3, 128, 17, 8)
        xrep = X[:, :, :, :, cmap].transpose(1, 2, 3, 4, 5, 0)
        xsum = xrep.sum(axis=4)                               # (NI,4,3,128,8)
        xr = np.concatenate(
            [xrep.reshape(NI, NSUB, 3, 128, ZF), xsum], axis=-1
        ).astype(f32)                                         # (NI,4,3,128,144)
        xr_cores.append(np.ascontiguousarray(xr.reshape(NI * NSUB * 3, 128, XF)))

    # y0 via init MLP (host), vectorized: x (B, C) -> (B, HID)
    relu = lambda v: np.maximum(v, 0.0, dtype=f32)
    h = relu(tys[:, 0] @ iW0.T + ib0[None, :])
    for k in range(iWh.shape[0]):
        h = relu(h @ iWh[k].T + ibh[k][None, :])
    y0 = (h @ iWo.T + ibo[None, :]).astype(f32)               # (B, HID)

    # weights: c-major permutation of fWo rows: row' = c*64 + h <- row h*33+c
    perm = np.zeros(CP * HID, np.int64) - 1
    csrc = np.arange(C)
    for h_i in range(HID):
        perm[csrc * HID + h_i] = h_i * C + csrc
    fWo_cm = np.zeros((CP * HID, WID), f32)
    fbo_cm = np.zeros((CP * HID,), f32)
    valid = perm >= 0
    fWo_cm[valid] = fWo[perm[valid]]
    fbo_cm[valid] = fbo[perm[valid]]

    # fWoT chunks: [128(w), 2176(row')] = concat of fWo_cm[128q:128q+128].T
    fWoT = np.ascontiguousarray(
        np.concatenate([fWo_cm[128 * q:128 * (q + 1)].T for q in range(NCHUNK)],
                       axis=1)).astype(f32)                   # (128, 2176)

    # Frep[part, 8q+b] = exp(2*fbo_cm[128q+part])
    Frep = np.exp(2.0 * fbo_cm.reshape(NCHUNK, 128)).T        # (128, 17)
    Frep = np.repeat(Frep[:, :, None], 8, axis=2).reshape(128, ZF).astype(f32)

    # S matrices [128, 64]: S[p, h] = r * (p % 64 == h)
    S1 = np.zeros((128, HID), f32)
    S1[np.arange(128), np.arange(128) % HID] = 1.0
    S_all = np.concatenate([3.0 * S1, 1.5 * S1, S1], axis=1)  # (128, 192)

    return xr_cores, y0, fWoT, Frep, S_all


# ------------------------------------------------------------- device kernel

def _patch_act_tables():
    """Restrict Exp/Ln to their shared table set so bacc's
    insert_act_table_loads hoists a single ACT_TABLE_LOAD instead of
    alternating sets before every activation (measured 21 ms of loads)."""
    import concourse.bacc as bacc
    import concourse.hw_specs as hw_specs
    import concourse.mybir as mybir

    if getattr(bacc, "_act_tables_patched", False):
        return
    T = mybir.ActivationFunctionType
    orig = hw_specs.get_activation_tables

    def patched(arch):
        tabs = orig(arch)
        for name, s in tabs.items():
            if name != "natural_log_exp_and_others":
                s.discard(T.Exp)
                s.discard(T.Ln)
        return tabs

    bacc.get_activation_tables = patched
    bacc._act_tables_patched = True


def _build(use_frep=False):
    import concourse.bass as bass
    import concourse.bacc as bacc
    import concourse.mybir as mybir
    import concourse.tile as tile

    _patch_act_tables()
    AF = mybir.ActivationFunctionType
    ALU = mybir.AluOpType
    f32 = mybir.dt.float32
    wdt = mybir.dt.float16 if BF16_WEIGHTS else f32

    nc = bacc.Bacc("TRN2", num_devices=N_CORES)

    # DRAM I/O (per core)
    d_xr = nc.dram_tensor("xr", [NI * NSUB * 3, 128, XF], f32, kind="ExternalInput")
    d_y0 = nc.dram_tensor("y0T", [HID, BL], f32, kind="ExternalInput")
    d_fW0T = nc.dram_tensor("fW0T", [HID, WID], wdt, kind="ExternalInput")
    d_fWhT = nc.dram_tensor("fWhT", [WID, 3 * WID], wdt, kind="ExternalInput")
    d_fWoT = nc.dram_tensor("fWoT", [WID, NCHUNK * 128], wdt, kind="ExternalInput")
    d_b0 = nc.dram_tensor("fb0c", [WID, 1], f32, kind="ExternalInput")
    d_bh = nc.dram_tensor("fbhc", [WID, 3], f32, kind="ExternalInput")
    d_Frep = nc.dram_tensor("Frep", [128, ZF], f32, kind="ExternalInput")
    d_S = nc.dram_tensor("S_all", [128, 3 * HID], f32, kind="ExternalInput")
    d_ysol = nc.dram_tensor("ysol", [NI, HID, BL], f32, kind="ExternalOutput")

    with tile.TileContext(nc) as tc:
        with tc.tile_pool(name="const", bufs=1) as cst, \
             tc.tile_pool(name="xr", bufs=6) as xrp, \
             tc.tile_pool(name="h", bufs=2) as hp, \
             tc.tile_pool(name="big", bufs=2) as bigp, \
             tc.tile_pool(name="sm", bufs=2) as smp, \
             tc.tile_pool(name="yst", bufs=2) as ystp, \
             tc.tile_pool(name="ylive", bufs=1) as ylp, \
             tc.tile_pool(name="lay", bufs=2, space="PSUM") as layp, \
             tc.tile_pool(name="z", bufs=2, space="PSUM") as zp, \
             tc.tile_pool(name="st", bufs=2, space="PSUM") as stp, \
             tc.tile_pool(name="comb", bufs=2, space="PSUM") as combp:

            # ---- constants to SBUF
            fW0T_s = cst.tile([HID, WID], wdt)
            fWhT_s = cst.tile([WID, 3 * WID], wdt)
            fWoT_s = cst.tile([WID, NCHUNK * 128], wdt)
            b0_s = cst.tile([WID, 1], f32)
            bh_s = cst.tile([WID, 3], f32)
            Frep_s = cst.tile([128, ZF], f32)
            S_s = cst.tile([128, 3 * HID], f32)
            y_s = ylp.tile([HID, BL], f32)

            nc.sync.dma_start(fW0T_s[:, :], d_fW0T.ap()[:, :])
            nc.sync.dma_start(fWhT_s[:, :], d_fWhT.ap()[:, :])
            nc.sync.dma_start(fWoT_s[:, :], d_fWoT.ap()[:, :])
            nc.sync.dma_start(b0_s[:, :], d_b0.ap()[:, :])
            nc.sync.dma_start(bh_s[:, :], d_bh.ap()[:, :])
            nc.sync.dma_start(Frep_s[:, :], d_Frep.ap()[:, :])
            nc.sync.dma_start(S_s[:, :], d_S.ap()[:, :])
            nc.sync.dma_start(y_s[:, :], d_y0.ap()[:, :])

            warm = cst.tile([1, 1], f32)
            nc.scalar.activation(warm[:, :], b0_s[0:1, 0:1], AF.Exp)
            nc.scalar.activation(warm[:, :], warm[:, :], AF.Ln, bias=1.0)

            xr_flat = d_xr.ap()

            def eval_vf(xrt, rhs_y):
                """One vf evaluation. Returns zsum [128, BL] in SBUF."""
                # 4 MLP layers
                p0 = layp.tile([WID, BL], f32, tag="lay")
                nc.tensor.matmul(p0[:, :], fW0T_s[:, :], rhs_y, start=True, stop=True)
                e0 = hp.tile([WID, BL], f32, tag="he")
                nc.scalar.activation(e0[:, :], p0[:, :], AF.Exp, bias=b0_s[:, 0:1])
                h = hp.tile([WID, BL], wdt, tag="hh")
                nc.scalar.activation(h[:, :], e0[:, :], AF.Ln, bias=1.0)
                for l in range(3):
                    pl = layp.tile([WID, BL], f32, tag="lay")
                    nc.tensor.matmul(pl[:, :], fWhT_s[:, 128 * l:128 * (l + 1)],
                                     h[:, :], start=True, stop=True)
                    el = hp.tile([WID, BL], f32, tag="he")
                    nc.scalar.activation(el[:, :], pl[:, :], AF.Exp,
                                         bias=bh_s[:, l:l + 1])
                    h = hp.tile([WID, BL], wdt, tag="hh")
                    nc.scalar.activation(h[:, :], el[:, :], AF.Ln, bias=1.0)

                # big matmul: z chunks [128, 136]
                zps = zp.tile([128, ZF], f32, tag="z")
                for q in range(NCHUNK):
                    nc.tensor.matmul(zps[:, 8 * q:8 * (q + 1)],
                                     fWoT_s[:, 128 * q:128 * (q + 1)],
                                     h[:, :], start=True, stop=True,
                                     skip_group_check=True)

                # head: E=exp(2z); d = E*Frep + 1; qd = xrep/d
                E = bigp.tile([128, ZF], f32, tag="E")
                nc.scalar.activation(E[:, :], zps[:, :], AF.Exp, scale=2.0)
                # dd = min(E*Frep + 1, 1e30): +1 for the sigmoid denom, clamp
                # so exp-overflow inf stays in reciprocal_approx_fast's domain
                dd = bigp.tile([128, ZF], f32, tag="dd")
                if use_frep:
                    nc.vector.tensor_tensor(dd[:, :], E[:, :], Frep_s[:, :],
                                            op=ALU.mult)
                    nc.vector.tensor_scalar(dd[:, :], dd[:, :], 1.0, 1e30,
                                            op0=ALU.add, op1=ALU.min)
                else:
                    nc.vector.tensor_scalar(dd[:, :], E[:, :], 1.0, 1e30,
                                            op0=ALU.add, op1=ALU.min)
                rr = bigp.tile([128, ZF], f32, tag="rr")
                nc.vector.reciprocal_approx_fast(rr[:, :], dd[:, :])
                qd = bigp.tile([128, ZF], f32, tag="qd")
                nc.vector.tensor_tensor(qd[:, :], xrt[:, 0:ZF], rr[:, :],
                                        op=ALU.mult)
                # rq = sum_q qd ; zsum = xrepsum - 2 rq
                rq = smp.tile([128, BL], f32, tag="rq")
                nc.vector.tensor_reduce(
                    rq[:, :],
                    qd[:, :].rearrange("p (q b) -> p b q", q=NCHUNK),
                    axis=mybir.AxisListType.X, op=ALU.add)
                zsum = smp.tile([128, BL], f32, tag="zsum")
                nc.vector.scalar_tensor_tensor(
                    zsum[:, :], rq[:, :], -2.0, xrt[:, ZF:XF],
                    op0=ALU.mult, op1=ALU.add)
                return zsum

            hints = (mybir.EngineType.PE, mybir.EngineType.Activation,
                     mybir.EngineType.DVE, mybir.EngineType.SP)
            with tc.For_i(0, NI, 1, hint_engines=hints,
                          staggered_reset=STAGGERED) as iv:
                for k in range(NSUB):
                    if STAGGERED and k > 0:
                        tc.stage_boundary()
                    base = iv * (NSUB * 3) + k * 3
                    xr0 = xrp.tile([128, XF], f32, tag="xr")
                    xr1 = xrp.tile([128, XF], f32, tag="xr")
                    xr2 = xrp.tile([128, XF], f32, tag="xr")
                    nc.sync.dma_start(xr0[:, :], xr_flat[bass.DynSlice(base, 1), :, :])
                    nc.sync.dma_start(xr1[:, :], xr_flat[bass.DynSlice(base + 1, 1), :, :])
                    nc.sync.dma_start(xr2[:, :], xr_flat[bass.DynSlice(base + 2, 1), :, :])
                    xrts = [xr0, xr1, xr1, xr2]
                    scol = [0, 64, 0, None]  # S3, S1.5, S3 col offsets

                    comb = combp.tile([HID, BL], f32, tag="comb")
                    y_bf = ystp.tile([HID, BL], wdt, tag="ybf")
                    nc.vector.tensor_copy(y_bf[:, :], y_s[:, :])
                    ystage = None
                    for j in range(4):
                        rhs = y_bf[:, :] if j == 0 else ystage[:, :]
                        zsum = eval_vf(xrts[j], rhs)
                        if j < 3:
                            st = stp.tile([HID, BL], f32, tag="st")
                            nc.tensor.matmul(st[:, :],
                                             S_s[:, scol[j]:scol[j] + HID],
                                             zsum[:, :], start=True, stop=True,
                                             skip_group_check=True)
                            ystage = ystp.tile([HID, BL], wdt, tag="yst")
                            nc.vector.tensor_tensor(ystage[:, :], y_s[:, :],
                                                    st[:, :], op=ALU.add)
                        nc.tensor.matmul(comb[:, :], S_s[:, 128:128 + HID],
                                         zsum[:, :], start=(j == 0),
                                         stop=(j == 3), skip_group_check=True)
                    nc.vector.tensor_tensor(y_s[:, :], y_s[:, :], comb[:, :],
                                            op=ALU.add)
                nc.sync.dma_start(d_ysol.ap()[bass.DynSlice(iv, 1), :, :], y_s[:, :])

    nc.compile()
    return nc


# ----------------------------------------------------------------- interface

def kernel(ts, ys, iW0, ib0, iWh, ibh, iWo, ibo, fW0, fb0, fWh, fbh, fWo, fbo,
           lW, lb):
    from concourse import bass_utils

    f32 = np.float32
    to_np = lambda a: np.asarray(a, dtype=f32)
    ts, ys = to_np(ts), to_np(ys)
    iW0, ib0, iWh, ibh = to_np(iW0), to_np(ib0), to_np(iWh), to_np(ibh)
    iWo, ibo = to_np(iWo), to_np(ibo)
    fW0, fb0, fWh, fbh = to_np(fW0), to_np(fb0), to_np(fWh), to_np(fbh)
    fWo, fbo, lW, lb = to_np(fWo), to_np(fbo), to_np(lW), to_np(lb)

    xr_cores, y0, fWoT, Frep, S_all = _host_precompute(
        ts, ys, iW0, ib0, iWh, ibh, iWo, ibo, fWo, fbo)

    use_frep = not (ASSUME_FBO_ZERO and not np.any(fbo))
    global _COMPILED
    if _COMPILED is None or _COMPILED[0] != use_frep:
        _COMPILED = (use_frep, _build(use_frep=use_frep))
    nc = _COMPILED[1]

    fW0T = np.ascontiguousarray(fW0.T)            # (64, 128)
    fWhT = np.ascontiguousarray(
        np.concatenate([fWh[k].T for k in range(3)], axis=1))  # (128, 384)
    if BF16_WEIGHTS:
        fW0T = fW0T.astype(np.float16)
        fWhT = fWhT.astype(np.float16)
        fWoT = fWoT.astype(np.float16)

    in_maps = []
    for core in range(N_CORES):
        sl = slice(core * BL, (core + 1) * BL)
        in_maps.append({
            "xr": xr_cores[core],
            "y0T": np.ascontiguousarray(y0[sl].T),
            "fW0T": fW0T,
            "fWhT": fWhT,
            "fWoT": fWoT,
            "fb0c": fb0[:, None],
            "fbhc": np.ascontiguousarray(fbh.T),
            "Frep": Frep,
            "S_all": S_all,
        })

    global _LAST_IN_MAPS
    _LAST_IN_MAPS = in_maps
    res = bass_utils.run_bass_kernel_spmd(nc, in_maps, core_ids=list(range(N_CORES)))

    ysol = np.empty((B, T, HID), f32)
    for core in range(N_CORES):
        sl = slice(core * BL, (core + 1) * BL)
        ysol[sl, 0] = y0[sl]
        ysol[sl, 1:] = res.results[core]["ysol"].transpose(2, 0, 1)

    out = ysol @ lW.T + lb[None, None, :]
    return out.astype(f32)


if __name__ == "__main__":
    pass



# revision 4
# speedup vs baseline: 2.4258x; 2.4258x over previous
"""NeuralCDE forward on 8 Trainium2 NeuronCores — v2.

The reference integrates with RK4 x 4 substeps (16 MLP evals/interval).
The wall-clock is bound by the *serial* eval chain (batch width is nearly
free), so v2:

1. Integrates with DOPRI5 + FSAL: 6 evals/interval (vs 16), validated
   rel_err ~1.8e-3 vs the reference (gate 2e-2).
2. Shortens each eval's chain by linearity-folding the stage combines:
   the stage state y_j is never materialized on-chain. The first-layer
   pre-activation u1_j = W0 @ y_j decomposes as
     u1_j = B_n + xsA_j + sum_m (-2 a_jm) * (W0.fold) @ rq_m
   where rq_m is the per-eval head reduction, B_n = W0 @ y_n carries via
   B_{n+1} = u1_7 (dopri5's 7th stage state IS y_{n+1}), and xsA_j is a
   host-precomputed rank-1 term. All combines are PE matmuls with
   pre-scaled stationaries (M0a); the old S-matmul/DVE stage tail is gone.
3. Streams rq out; the host reconstructs y (K_m = S_m - 2*fold(rq_m)) and
   applies the readout. fbo == 0 assumed (checked; Frep fallback built
   on demand as in the baseline).
4. Head is split into two chunk groups (9+8) so Exp/DVE overlap the
   chunk matmuls; softplus stays Exp+Ln on ACT (single act-table set).
"""

import numpy as np

N_CORES = 8
T = 128
B = 64
OBS = 32
HID = 64
WID = 128
OUT = 32
C = OBS + 1          # 33
CP = 34              # padded C (even)
NCHUNK = 17          # 2176 / 128
ZF = NCHUNK * 8      # 136
XF = ZF + 8          # 144: xrep ++ xsA
NI = T - 1           # 127 intervals
NST = 6              # dopri5 evals per interval (stages 2..7)
NEV = 1 + NI * NST   # total evals incl. initial k1
BL = B // N_CORES    # 8 per core
NQA = 9              # chunks in head group A
NQB = NCHUNK - NQA   # 8
ZA = NQA * 8         # 72
STAGGERED = True

_COMPILED = None
_LAST_IN_MAPS = None

# dopri5 tableau (row 7 = b; FSAL)
_A = np.zeros((8, 8))
_A[2, 1] = 1 / 5
_A[3, 1:3] = [3 / 40, 9 / 40]
_A[4, 1:4] = [44 / 45, -56 / 15, 32 / 9]
_A[5, 1:5] = [19372 / 6561, -25360 / 2187, 64448 / 6561, -212 / 729]
_A[6, 1:6] = [9017 / 3168, -355 / 33, 46732 / 5247, 49 / 176, -5103 / 18656]
_A[7, 1:7] = [35 / 384, 0.0, 500 / 1113, 125 / 192, -2187 / 6784, 11 / 84]
_CS = [0.0, 0.0, 1 / 5, 3 / 10, 4 / 5, 8 / 9, 1.0, 1.0]
# (j, m) pairs with a_jm != 0, in emission order per stage
_JM = [(j, m) for j in range(2, 8) for m in range(1, j) if _A[j, m] != 0.0]
NM = len(_JM)        # 20


# ----------------------------------------------------------------- host math

def _host_precompute(ts, ys, iW0, ib0, iWh, ibh, iWo, ibo, fW0, fWo):
    f32 = np.float32
    ts = ts.astype(f32)
    ys = ys.astype(f32)

    tys = np.concatenate([np.broadcast_to(ts[None, :, None], (B, T, 1)), ys], axis=-1)
    dts = ts[1:] - ts[:-1]
    diffs = (tys[:, 1:] - tys[:, :-1]) / dts[None, :, None]
    deriv = np.concatenate([diffs[:, :1], diffs], axis=1)
    d0 = deriv[:, :-1]
    d1 = deriv[:, 1:]
    cc = (3.0 * diffs - 2.0 * d0 - d1) / dts[None, :, None]
    bb = (d0 + d1 - 2.0 * diffs) / (dts * dts)[None, :, None]

    # X[b, i, jj, c] = h * xdot at stage (jj+2)'s c-point; X0 = initial c=0
    cpts = np.array([_CS[j] for j in range(2, 8)], f32)
    s = (cpts[None, :] * dts[:, None])[None, :, :, None]
    X = (d0[:, :, None, :] + 2.0 * cc[:, :, None, :] * s
         + 3.0 * bb[:, :, None, :] * s * s) * dts[None, :, None, None]
    X = X.astype(f32)                                  # (B, NI, 6, C)
    X0 = (d0[:, 0] * dts[0]).astype(f32)               # (B, C)

    S_all = X.sum(-1)                                  # (B, NI, 6)
    S0 = X0.sum(-1)                                    # (B,)
    rowsumW0 = fW0.sum(axis=1).astype(f32)             # (128,)

    # y0 via init MLP
    relu = lambda v: np.maximum(v, 0.0, dtype=f32)
    h = relu(tys[:, 0] @ iW0.T + ib0[None, :])
    for k in range(iWh.shape[0]):
        h = relu(h @ iWh[k].T + ibh[k][None, :])
    y0 = (h @ iWo.T + ibo[None, :]).astype(f32)        # (B, HID)

    # xsA scalars per (i, jj): sum_{m<j} a_jm * S_m  -> (B, NI, 6)
    xsA_s = np.zeros((B, NI, NST), f32)
    for jj in range(NST):
        j = jj + 2
        for m in range(1, j):
            a = _A[j, m]
            if a == 0.0:
                continue
            if m == 1:
                Sm = np.concatenate([S0[:, None], S_all[:, :-1, 5]], axis=1)  # (B, NI)
            else:
                Sm = S_all[:, :, m - 2]
            xsA_s[:, :, jj] += np.float32(a) * Sm

    # per-core xr tiles: [NEV(+pad), 128, XF]: xrep cols 0:136, xsA cols 136:144
    q_idx = np.arange(NCHUNK)
    part_half = np.arange(128) // 64
    cmap = (2 * q_idx[None, :] + part_half[:, None])   # (128, 17)

    xr_cores = []
    for core in range(N_CORES):
        sl = slice(core * BL, (core + 1) * BL)
        Xp = np.zeros((BL, NI, NST, CP), f32)
        Xp[..., :C] = X[sl]
        xr = Xp[:, :, :, cmap]                         # (BL, NI, 6, 128, 17)
        xr = xr.transpose(1, 2, 3, 4, 0).reshape(NI * NST, 128, ZF)
        xsA = rowsumW0[None, :, None] * xsA_s[sl].transpose(1, 2, 0).reshape(
            NI * NST, 1, BL)                           # (NI*6, 128, BL)
        tiles = np.zeros((NEV + NST, 128, XF), f32)    # +NST zero-pad tiles
        tiles[1:NEV, :, :ZF] = xr
        tiles[1:NEV, :, ZF:] = xsA
        X0p = np.zeros((BL, CP), f32)
        X0p[:, :C] = X0[sl]
        tiles[0, :, :ZF] = X0p[:, cmap].transpose(1, 2, 0).reshape(128, ZF)
        xr_cores.append(np.ascontiguousarray(tiles))

    # M0 stationaries (lhsT layout): M0a_{jm} = (-2 a_jm) * W0F, W0F[o,p]=W0[o,p%64]
    W0F = np.concatenate([fW0, fW0], axis=1).astype(f32)      # (128, 128)
    mats = [np.ascontiguousarray((np.float32(-2.0 * _A[j, m]) * W0F).T)
            for (j, m) in _JM]
    mats.append(np.eye(128, dtype=f32))                       # identity last
    M0cat = np.concatenate(mats, axis=1).astype(f32)          # (128, (NM+1)*128)

    B0 = np.stack([np.ascontiguousarray(fW0 @ y0[c * BL:(c + 1) * BL].T)
                   for c in range(N_CORES)])                  # (ncores, 128, BL)

    return xr_cores, M0cat, B0, y0, S_all, S0


def _host_reconstruct(rq_cores, y0, S_all, S0, lW, lb):
    f32 = np.float32
    # stack cores on batch axis: rq_full [NEV, 128, B]
    rq = np.concatenate([rq_cores[c] for c in range(N_CORES)], axis=2).astype(f32)
    K = -2.0 * (rq[:, :HID, :] + rq[:, HID:, :]).transpose(0, 2, 1)  # (NEV, B, HID)
    # add S_m per eval
    K[0] += S0[:, None]
    K[1:] += S_all.reshape(B, NI * NST).T[:, :, None]
    bvec = _A[7]
    ysol = np.zeros((B, T, HID), f32)
    ysol[:, 0] = y0
    y = y0.copy()
    for i in range(NI):
        k1 = K[0] if i == 0 else K[1 + (i - 1) * NST + 5]
        dy = np.float32(bvec[1]) * k1
        for m in range(3, 8):        # b2 == 0
            dy += np.float32(bvec[m]) * K[1 + i * NST + (m - 2)]
        y = y + dy
        ysol[:, i + 1] = y
    return ysol


# ------------------------------------------------------------- device kernel

def _patch_act_tables():
    """Keep Exp/Ln only in their shared table set so a single
    ACT_TABLE_LOAD is hoisted (see baseline)."""
    import concourse.bacc as bacc
    import concourse.hw_specs as hw_specs
    import concourse.mybir as mybir

    if getattr(bacc, "_act_tables_patched", False):
        return
    Tt = mybir.ActivationFunctionType
    orig = hw_specs.get_activation_tables

    def patched(arch):
        tabs = orig(arch)
        for name, s in tabs.items():
            if name != "natural_log_exp_and_others":
                s.discard(Tt.Exp)
                s.discard(Tt.Ln)
        return tabs

    bacc.get_activation_tables = patched
    bacc._act_tables_patched = True


def _build(use_frep=False):
    import concourse.bass as bass
    import concourse.bacc as bacc
    import concourse.mybir as mybir
    import concourse.tile as tile

    _patch_act_tables()
    AF = mybir.ActivationFunctionType
    ALU = mybir.AluOpType
    f32 = mybir.dt.float32
    f16 = mybir.dt.float16

    nc = bacc.Bacc("TRN2", num_devices=N_CORES)

    d_xr = nc.dram_tensor("xr", [NEV + NST, 128, XF], f32, kind="ExternalInput")
    d_M0 = nc.dram_tensor("M0cat", [128, (NM + 1) * 128], f32, kind="ExternalInput")
    d_B0 = nc.dram_tensor("B0", [128, BL], f32, kind="ExternalInput")
    d_fWhT = nc.dram_tensor("fWhT", [WID, 3 * WID], f16, kind="ExternalInput")
    d_fWoT = nc.dram_tensor("fWoT", [WID, NCHUNK * 128], f16, kind="ExternalInput")
    d_b0 = nc.dram_tensor("fb0c", [WID, 1], f32, kind="ExternalInput")
    d_bh = nc.dram_tensor("fbhc", [WID, 3], f32, kind="ExternalInput")
    d_Frep = nc.dram_tensor("Frep", [128, ZF], f32, kind="ExternalInput")
    d_rq = nc.dram_tensor("rq", [NEV, 128, BL], f32, kind="ExternalOutput")

    # M0 slices: index by (j, m)
    m0_col = {jm: 128 * k for k, jm in enumerate(_JM)}
    id_col = 128 * NM

    with tile.TileContext(nc) as tc:
        with tc.tile_pool(name="const", bufs=1) as cst, \
             tc.tile_pool(name="xr", bufs=1) as xrp, \
             tc.tile_pool(name="h", bufs=2) as hp, \
             tc.tile_pool(name="big", bufs=2) as bigp, \
             tc.tile_pool(name="rqs", bufs=1) as rqp, \
             tc.tile_pool(name="dd", bufs=2) as ddp, \
             tc.tile_pool(name="u1", bufs=2, space="PSUM") as u1p, \
             tc.tile_pool(name="lay", bufs=2, space="PSUM") as layp, \
             tc.tile_pool(name="za", bufs=2, space="PSUM") as zap, \
             tc.tile_pool(name="zb", bufs=2, space="PSUM") as zbp:

            M0_s = cst.tile([128, (NM + 1) * 128], f32)
            fWhT_s = cst.tile([WID, 3 * WID], f16)
            fWoT_s = cst.tile([WID, NCHUNK * 128], f16)
            b0_s = cst.tile([WID, 1], f32)
            bh_s = cst.tile([WID, 3], f32)
            B_s = cst.tile([128, BL], f32)       # base carry W0 @ y_n
            rq17_s = cst.tile([128, BL], f32)    # FSAL carry rq_1 / rq_7
            Frep_s = cst.tile([128, ZF], f32)

            nc.sync.dma_start(M0_s[:, :], d_M0.ap()[:, :])
            nc.sync.dma_start(fWhT_s[:, :], d_fWhT.ap()[:, :])
            nc.sync.dma_start(fWoT_s[:, :], d_fWoT.ap()[:, :])
            nc.sync.dma_start(b0_s[:, :], d_b0.ap()[:, :])
            nc.sync.dma_start(bh_s[:, :], d_bh.ap()[:, :])
            nc.sync.dma_start(B_s[:, :], d_B0.ap()[:, :])
            if use_frep:
                nc.sync.dma_start(Frep_s[:, :], d_Frep.ap()[:, :])

            warm = cst.tile([1, 1], f32)
            nc.scalar.activation(warm[:, :], b0_s[0:1, 0:1], AF.Exp)
            nc.scalar.activation(warm[:, :], warm[:, :], AF.Ln, bias=1.0)

            xr_flat = d_xr.ap()

            def eval_chain(u1ps, xrt, rq_out):
                """u1ps: assembled PSUM [128, BL]. Emits the MLP + head;
                writes rq into rq_out [128, BL] f32 SBUF."""
                # layer 0 softplus: Exp then Ln (ACT), h in fp16 for the MMs
                e0 = hp.tile([WID, BL], f32, tag="e", bufs=2)
                nc.scalar.activation(e0[:, :], u1ps[:, :], AF.Exp, bias=b0_s[:, 0:1])
                h = hp.tile([WID, BL], f16, tag="h", bufs=3)
                nc.scalar.activation(h[:, :], e0[:, :], AF.Ln, bias=1.0)
                for l in range(3):
                    pl = layp.tile([WID, BL], f32, tag="lay")
                    nc.tensor.matmul(pl[:, :], fWhT_s[:, 128 * l:128 * (l + 1)],
                                     h[:, :], start=True, stop=True)
                    el = hp.tile([WID, BL], f32, tag="e", bufs=2)
                    nc.scalar.activation(el[:, :], pl[:, :], AF.Exp,
                                         bias=bh_s[:, l:l + 1])
                    h = hp.tile([WID, BL], f16, tag="h", bufs=3)
                    nc.scalar.activation(h[:, :], el[:, :], AF.Ln, bias=1.0)

                # chunk matmuls, two groups
                zA = zap.tile([128, ZA], f32, tag="za")
                zB = zbp.tile([128, ZF - ZA], f32, tag="zb")
                for q in range(NQA):
                    nc.tensor.matmul(zA[:, 8 * q:8 * (q + 1)],
                                     fWoT_s[:, 128 * q:128 * (q + 1)],
                                     h[:, :], start=True, stop=True,
                                     skip_group_check=True)
                for q in range(NQB):
                    qq = NQA + q
                    nc.tensor.matmul(zB[:, 8 * q:8 * (q + 1)],
                                     fWoT_s[:, 128 * qq:128 * (qq + 1)],
                                     h[:, :], start=True, stop=True,
                                     skip_group_check=True)

                # head per group: E=exp(2z); rr=1/(E+1); qd=xrep*rr; reduce
                rqg = []
                for g, (zps, cols, nq) in enumerate(
                        [(zA, slice(0, ZA), NQA), (zB, slice(ZA, ZF), NQB)]):
                    w = nq * 8
                    E = bigp.tile([128, w], f32, tag=f"E{g}")
                    nc.scalar.activation(E[:, :], zps[:, :], AF.Exp, scale=2.0)
                    dd = ddp.tile([128, w], f32, tag=f"dd{g}")
                    if use_frep:
                        nc.vector.tensor_tensor(dd[:, :], E[:, :],
                                                Frep_s[:, cols], op=ALU.mult)
                        nc.vector.tensor_scalar(dd[:, :], dd[:, :], 1.0, 1e30,
                                                op0=ALU.add, op1=ALU.min)
                    else:
                        nc.vector.tensor_scalar(dd[:, :], E[:, :], 1.0, 1e30,
                                                op0=ALU.add, op1=ALU.min)
                    rr = ddp.tile([128, w], f32, tag=f"rr{g}")
                    nc.vector.reciprocal_approx_fast(rr[:, :], dd[:, :])
                    qd = bigp.tile([128, w], f32, tag=f"qd{g}")
                    nc.vector.tensor_tensor(qd[:, :], xrt[:, cols], rr[:, :],
                                            op=ALU.mult)
                    rg = rqp.tile([128, BL], f32, tag=f"rg{g}", bufs=2)
                    nc.vector.tensor_reduce(
                        rg[:, :],
                        qd[:, :].rearrange("p (q b) -> p b q", q=nq),
                        axis=mybir.AxisListType.X, op=ALU.add)
                    rqg.append(rg)
                nc.vector.tensor_tensor(rq_out[:, :], rqg[0][:, :], rqg[1][:, :],
                                        op=ALU.add)

            # ---------------- pre-loop: initial k1 eval ----------------
            xr0 = xrp.tile([128, XF], f32, tag="xr0")
            nc.sync.dma_start(xr0[:, :], xr_flat[bass.DynSlice(0, 1), :, :])
            u1i = u1p.tile([128, BL], f32, tag="u1")
            nc.tensor.matmul(u1i[:, :], M0_s[:, id_col:id_col + 128], B_s[:, :],
                             start=True, stop=True, skip_group_check=True)
            eval_chain(u1i, xr0, rq17_s)
            nc.sync.dma_start(d_rq.ap()[bass.DynSlice(0, 1), :, :], rq17_s[:, :])

            # prime xr tiles for slots 0, 1 (indices 1, 2)
            slotread = [xrp.tile([128, XF], f32, tag=f"xrs{s}", name=f"xrs{s}")
                        for s in range(NST)]
            nc.sync.dma_start(slotread[0][:, :], xr_flat[bass.DynSlice(1, 1), :, :])
            nc.sync.dma_start(slotread[1][:, :], xr_flat[bass.DynSlice(2, 1), :, :])

            # D_2 for interval 0: B + xsA(tile of slot 0)
            D2_s = cst.tile([128, BL], f32)
            nc.vector.tensor_tensor(D2_s[:, :], B_s[:, :], slotread[0][:, ZF:XF],
                                    op=ALU.add)

            # rq_m tile registry: m=1 -> rq17_s; m=2..6 -> per-slot pool tiles
            rq_tiles = {1: rq17_s}

            hints = (mybir.EngineType.PE, mybir.EngineType.Activation,
                     mybir.EngineType.DVE, mybir.EngineType.SP)
            with tc.For_i(0, NI, 1, hint_engines=hints,
                          staggered_reset=STAGGERED) as iv:
                for s in range(NST):
                    j = s + 2

                    # u1_j assembly: id-MM(D_j) + off-chain M0a terms + chain term
                    u1 = u1p.tile([128, BL], f32, tag="u1")
                    Dj = D2_s if s == 0 else rq_tiles[("D", j)]
                    nc.tensor.matmul(u1[:, :], M0_s[:, id_col:id_col + 128],
                                     Dj[:, :], start=True, stop=False,
                                     skip_group_check=True)
                    for m in range(1, j - 1):
                        if _A[j, m] == 0.0:
                            continue
                        col = m0_col[(j, m)]
                        nc.tensor.matmul(u1[:, :], M0_s[:, col:col + 128],
                                         rq_tiles[m][:, :], start=False,
                                         stop=False, skip_group_check=True)
                    col = m0_col[(j, j - 1)]
                    nc.tensor.matmul(u1[:, :], M0_s[:, col:col + 128],
                                     rq_tiles[j - 1][:, :], start=False,
                                     stop=True, skip_group_check=True)

                    # target rq tile for this stage
                    if j == 7:
                        rq_out = rq17_s
                    else:
                        rq_out = rqp.tile([128, BL], f32, tag=f"rq_{s}", bufs=1)
                        rq_tiles[j] = rq_out

                    eval_chain(u1, slotread[s], rq_out)

                    nc.sync.dma_start(
                        d_rq.ap()[bass.DynSlice(iv * NST + (s + 1), 1), :, :],
                        rq_out[:, :])

                    # prefetch xr for slot s+2 (rolls into next interval)
                    nc.sync.dma_start(
                        slotread[(s + 2) % NST][:, :],
                        xr_flat[bass.DynSlice(iv * NST + (s + 3), 1), :, :])

                    # D for stage j+1 (slots 0..4) or next interval's D_2 (slot 5)
                    if s < NST - 1:
                        Dn = rqp.tile([128, BL], f32, tag=f"D_{s}", bufs=1)
                        rq_tiles[("D", j + 1)] = Dn
                        nc.vector.tensor_tensor(Dn[:, :], B_s[:, :],
                                                slotread[s + 1][:, ZF:XF],
                                                op=ALU.add)
                    else:
                        # B_{n+1} = u1_7 content (dopri5: stage-7 state = y_{n+1})
                        nc.vector.tensor_copy(B_s[:, :], u1[:, :])
                        nc.vector.tensor_tensor(D2_s[:, :], B_s[:, :],
                                                slotread[0][:, ZF:XF],
                                                op=ALU.add)

    nc.compile()
    return nc


# ----------------------------------------------------------------- interface

def kernel(ts, ys, iW0, ib0, iWh, ibh, iWo, ibo, fW0, fb0, fWh, fbh, fWo, fbo,
           lW, lb):
    from concourse import bass_utils

    f32 = np.float32
    to_np = lambda a: np.asarray(a, dtype=f32)
    ts, ys = to_np(ts), to_np(ys)
    iW0, ib0, iWh, ibh = to_np(iW0), to_np(ib0), to_np(iWh), to_np(ibh)
    iWo, ibo = to_np(iWo), to_np(ibo)
    fW0, fb0, fWh, fbh = to_np(fW0), to_np(fb0), to_np(fWh), to_np(fbh)
    fWo, fbo, lW, lb = to_np(fWo), to_np(fbo), to_np(lW), to_np(lb)

    xr_cores, M0cat, B0, y0, S_all, S0 = _host_precompute(
        ts, ys, iW0, ib0, iWh, ibh, iWo, ibo, fW0, fWo)

    # c-major permuted fWo (baseline layout) + Frep fallback
    perm = np.zeros(CP * HID, np.int64) - 1
    csrc = np.arange(C)
    for h_i in range(HID):
        perm[csrc * HID + h_i] = h_i * C + csrc
    fWo_cm = np.zeros((CP * HID, WID), f32)
    fbo_cm = np.zeros((CP * HID,), f32)
    valid = perm >= 0
    fWo_cm[valid] = fWo[perm[valid]]
    fbo_cm[valid] = fbo[perm[valid]]
    fWoT = np.ascontiguousarray(
        np.concatenate([fWo_cm[128 * q:128 * (q + 1)].T for q in range(NCHUNK)],
                       axis=1)).astype(np.float16)
    Frep = np.exp(2.0 * fbo_cm.reshape(NCHUNK, 128)).T
    Frep = np.repeat(Frep[:, :, None], 8, axis=2).reshape(128, ZF).astype(f32)

    use_frep = bool(np.any(fbo))
    global _COMPILED
    if _COMPILED is None or _COMPILED[0] != use_frep:
        _COMPILED = (use_frep, _build(use_frep=use_frep))
    nc = _COMPILED[1]

    fWhT = np.ascontiguousarray(
        np.concatenate([fWh[k].T for k in range(3)], axis=1)).astype(np.float16)

    in_maps = []
    for core in range(N_CORES):
        in_maps.append({
            "xr": xr_cores[core],
            "M0cat": M0cat,
            "B0": B0[core],
            "fWhT": fWhT,
            "fWoT": fWoT,
            "fb0c": fb0[:, None],
            "fbhc": np.ascontiguousarray(fbh.T),
            "Frep": Frep,
        })

    global _LAST_IN_MAPS
    _LAST_IN_MAPS = in_maps
    res = bass_utils.run_bass_kernel_spmd(nc, in_maps, core_ids=list(range(N_CORES)))

    rq_cores = [res.results[core]["rq"] for core in range(N_CORES)]
    ysol = _host_reconstruct(rq_cores, y0, S_all, S0, lW, lb)
    out = ysol @ lW.T + lb[None, None, :]
    return out.astype(f32)


if __name__ == "__main__":
    pass


# revision 6
# speedup vs baseline: 2.8685x; 1.1825x over previous
"""NeuralCDE forward on 8 Trainium2 NeuronCores — v2.

The reference integrates with RK4 x 4 substeps (16 MLP evals/interval).
The wall-clock is bound by the *serial* eval chain (batch width is nearly
free), so v2:

1. Integrates with DOPRI5 + FSAL: 6 evals/interval (vs 16), validated
   rel_err ~1.8e-3 vs the reference (gate 2e-2).
2. Shortens each eval's chain by linearity-folding the stage combines:
   the stage state y_j is never materialized on-chain. The first-layer
   pre-activation u1_j = W0 @ y_j decomposes as
     u1_j = B_n + xsA_j + sum_m (-2 a_jm) * (W0.fold) @ rq_m
   where rq_m is the per-eval head reduction, B_n = W0 @ y_n carries via
   B_{n+1} = u1_7 (dopri5's 7th stage state IS y_{n+1}), and xsA_j is a
   host-precomputed rank-1 term. All combines are PE matmuls with
   pre-scaled stationaries (M0a); the old S-matmul/DVE stage tail is gone.
3. Streams rq out; the host reconstructs y (K_m = S_m - 2*fold(rq_m)) and
   applies the readout. fbo == 0 assumed (checked; Frep fallback built
   on demand as in the baseline).
4. Head is split into two chunk groups (9+8) so Exp/DVE overlap the
   chunk matmuls; softplus stays Exp+Ln on ACT (single act-table set).
"""

import numpy as np

N_CORES = 8
T = 128
B = 64
OBS = 32
HID = 64
WID = 128
OUT = 32
C = OBS + 1          # 33
CP = 34              # padded C (even)
NCHUNK = 17          # 2176 / 128
ZF = NCHUNK * 8      # 136
XF = ZF + 8          # 144: xrep ++ xsA
NI = T - 1           # 127 intervals
NST = 6              # dopri5 evals per interval (stages 2..7)
NEV = 1 + NI * NST   # total evals incl. initial k1
BL = B // N_CORES    # 8 per core
NQA = 9              # chunks in head group A
NQB = NCHUNK - NQA   # 8
ZA = NQA * 8         # 72
STAGGERED = True

_COMPILED = None
_LAST_IN_MAPS = None

# dopri5 tableau (row 7 = b; FSAL)
_A = np.zeros((8, 8))
_A[2, 1] = 1 / 5
_A[3, 1:3] = [3 / 40, 9 / 40]
_A[4, 1:4] = [44 / 45, -56 / 15, 32 / 9]
_A[5, 1:5] = [19372 / 6561, -25360 / 2187, 64448 / 6561, -212 / 729]
_A[6, 1:6] = [9017 / 3168, -355 / 33, 46732 / 5247, 49 / 176, -5103 / 18656]
_A[7, 1:7] = [35 / 384, 0.0, 500 / 1113, 125 / 192, -2187 / 6784, 11 / 84]
_CS = [0.0, 0.0, 1 / 5, 3 / 10, 4 / 5, 8 / 9, 1.0, 1.0]
# (j, m) pairs with a_jm != 0, in emission order per stage
_JM = [(j, m) for j in range(2, 8) for m in range(1, j) if _A[j, m] != 0.0]
NM = len(_JM)        # 20


# ----------------------------------------------------------------- host math

def _host_precompute(ts, ys, iW0, ib0, iWh, ibh, iWo, ibo, fW0, fWo):
    f32 = np.float32
    ts = ts.astype(f32)
    ys = ys.astype(f32)

    tys = np.concatenate([np.broadcast_to(ts[None, :, None], (B, T, 1)), ys], axis=-1)
    dts = ts[1:] - ts[:-1]
    diffs = (tys[:, 1:] - tys[:, :-1]) / dts[None, :, None]
    deriv = np.concatenate([diffs[:, :1], diffs], axis=1)
    d0 = deriv[:, :-1]
    d1 = deriv[:, 1:]
    cc = (3.0 * diffs - 2.0 * d0 - d1) / dts[None, :, None]
    bb = (d0 + d1 - 2.0 * diffs) / (dts * dts)[None, :, None]

    # X[b, i, jj, c] = h * xdot at stage (jj+2)'s c-point; X0 = initial c=0
    cpts = np.array([_CS[j] for j in range(2, 8)], f32)
    s = (cpts[None, :] * dts[:, None])[None, :, :, None]
    X = (d0[:, :, None, :] + 2.0 * cc[:, :, None, :] * s
         + 3.0 * bb[:, :, None, :] * s * s) * dts[None, :, None, None]
    X = X.astype(f32)                                  # (B, NI, 6, C)
    X0 = (d0[:, 0] * dts[0]).astype(f32)               # (B, C)

    S_all = X.sum(-1)                                  # (B, NI, 6)
    S0 = X0.sum(-1)                                    # (B,)
    rowsumW0 = fW0.sum(axis=1).astype(f32)             # (128,)

    # y0 via init MLP
    relu = lambda v: np.maximum(v, 0.0, dtype=f32)
    h = relu(tys[:, 0] @ iW0.T + ib0[None, :])
    for k in range(iWh.shape[0]):
        h = relu(h @ iWh[k].T + ibh[k][None, :])
    y0 = (h @ iWo.T + ibo[None, :]).astype(f32)        # (B, HID)

    # xsA scalars per (i, jj): sum_{m<j} a_jm * S_m  -> (B, NI, 6)
    xsA_s = np.zeros((B, NI, NST), f32)
    for jj in range(NST):
        j = jj + 2
        for m in range(1, j):
            a = _A[j, m]
            if a == 0.0:
                continue
            if m == 1:
                Sm = np.concatenate([S0[:, None], S_all[:, :-1, 5]], axis=1)  # (B, NI)
            else:
                Sm = S_all[:, :, m - 2]
            xsA_s[:, :, jj] += np.float32(a) * Sm

    # per-core xr tiles: [NEV(+pad), 128, XF]: xrep cols 0:136, xsA cols 136:144
    q_idx = np.arange(NCHUNK)
    part_half = np.arange(128) // 64
    cmap = (2 * q_idx[None, :] + part_half[:, None])   # (128, 17)

    xr_cores = []
    for core in range(N_CORES):
        sl = slice(core * BL, (core + 1) * BL)
        Xp = np.zeros((BL, NI, NST, CP), f32)
        Xp[..., :C] = X[sl]
        xr = Xp[:, :, :, cmap]                         # (BL, NI, 6, 128, 17)
        xr = xr.transpose(1, 2, 3, 4, 0).reshape(NI * NST, 128, ZF)
        xsA = rowsumW0[None, :, None] * xsA_s[sl].transpose(1, 2, 0).reshape(
            NI * NST, 1, BL)                           # (NI*6, 128, BL)
        tiles = np.zeros((NEV + NST, 128, XF), np.float16)  # +NST zero-pad
        tiles[1:NEV, :, :ZF] = xr
        tiles[1:NEV, :, ZF:] = xsA
        X0p = np.zeros((BL, CP), f32)
        X0p[:, :C] = X0[sl]
        tiles[0, :, :ZF] = X0p[:, cmap].transpose(1, 2, 0).reshape(128, ZF)
        xr_cores.append(np.ascontiguousarray(tiles))

    # M0 stationaries (lhsT layout): M0a_{jm} = (-2 a_jm) * W0F, W0F[o,p]=W0[o,p%64]
    W0F = np.concatenate([fW0, fW0], axis=1).astype(f32)      # (128, 128)
    mats = [np.ascontiguousarray((np.float32(-2.0 * _A[j, m]) * W0F).T)
            for (j, m) in _JM]
    mats.append(np.eye(128, dtype=f32))                       # identity last
    M0cat = np.concatenate(mats, axis=1).astype(np.float16)   # (128, (NM+1)*128)

    B0 = np.stack([np.ascontiguousarray(fW0 @ y0[c * BL:(c + 1) * BL].T)
                   for c in range(N_CORES)]).astype(np.float16)

    return xr_cores, M0cat, B0, y0, S_all, S0


def _host_reconstruct(rq_cores, y0, S_all, S0, lW, lb):
    f32 = np.float32
    # stack cores on batch axis: rq_full [NEV, 128, B]
    rq = np.concatenate([rq_cores[c] for c in range(N_CORES)], axis=2).astype(f32)
    K = -2.0 * (rq[:, :HID, :] + rq[:, HID:, :]).transpose(0, 2, 1)  # (NEV, B, HID)
    # add S_m per eval
    K[0] += S0[:, None]
    K[1:] += S_all.reshape(B, NI * NST).T[:, :, None]
    bvec = _A[7]
    ysol = np.zeros((B, T, HID), f32)
    ysol[:, 0] = y0
    y = y0.copy()
    for i in range(NI):
        k1 = K[0] if i == 0 else K[1 + (i - 1) * NST + 5]
        dy = np.float32(bvec[1]) * k1
        for m in range(3, 8):        # b2 == 0
            dy += np.float32(bvec[m]) * K[1 + i * NST + (m - 2)]
        y = y + dy
        ysol[:, i + 1] = y
    return ysol


# ------------------------------------------------------------- device kernel

def _patch_act_tables():
    """Keep Exp/Ln only in their shared table set so a single
    ACT_TABLE_LOAD is hoisted (see baseline)."""
    import concourse.bacc as bacc
    import concourse.hw_specs as hw_specs
    import concourse.mybir as mybir

    if getattr(bacc, "_act_tables_patched", False):
        return
    Tt = mybir.ActivationFunctionType
    orig = hw_specs.get_activation_tables

    def patched(arch):
        tabs = orig(arch)
        for name, s in tabs.items():
            if name != "natural_log_exp_and_others":
                s.discard(Tt.Exp)
                s.discard(Tt.Ln)
        return tabs

    bacc.get_activation_tables = patched
    bacc._act_tables_patched = True


def _build(use_frep=False):
    import concourse.bass as bass
    import concourse.bacc as bacc
    import concourse.mybir as mybir
    import concourse.tile as tile

    _patch_act_tables()
    AF = mybir.ActivationFunctionType
    ALU = mybir.AluOpType
    f32 = mybir.dt.float32
    f16 = mybir.dt.float16

    nc = bacc.Bacc("TRN2", num_devices=N_CORES)

    d_xr = nc.dram_tensor("xr", [NEV + NST, 128, XF], f16, kind="ExternalInput")
    d_M0 = nc.dram_tensor("M0cat", [128, (NM + 1) * 128], f16, kind="ExternalInput")
    d_B0 = nc.dram_tensor("B0", [128, BL], f16, kind="ExternalInput")
    d_fWhT = nc.dram_tensor("fWhT", [WID, 3 * WID], f16, kind="ExternalInput")
    d_fWoT = nc.dram_tensor("fWoT", [WID, NCHUNK * 128], f16, kind="ExternalInput")
    d_b0 = nc.dram_tensor("fb0c", [WID, 1], f32, kind="ExternalInput")
    d_bh = nc.dram_tensor("fbhc", [WID, 3], f32, kind="ExternalInput")
    d_Frep = nc.dram_tensor("Frep", [128, ZF], f32, kind="ExternalInput")
    d_rq = nc.dram_tensor("rq", [NEV, 128, BL], f16, kind="ExternalOutput")

    m0_col = {jm: 128 * k for k, jm in enumerate(_JM)}
    id_col = 128 * NM

    with tile.TileContext(nc) as tc, \
         nc.allow_low_precision("fp16 rq stream validated offline (4.7e-3)"):
        with tc.tile_pool(name="const", bufs=1) as cst, \
             tc.tile_pool(name="xr", bufs=1) as xrp, \
             tc.tile_pool(name="h", bufs=2) as hp, \
             tc.tile_pool(name="big", bufs=2) as bigp, \
             tc.tile_pool(name="rqs", bufs=1) as rqp, \
             tc.tile_pool(name="dd", bufs=2) as ddp, \
             tc.tile_pool(name="u1", bufs=2, space="PSUM") as u1p, \
             tc.tile_pool(name="lay", bufs=2, space="PSUM") as layp, \
             tc.tile_pool(name="za", bufs=2, space="PSUM") as zap, \
             tc.tile_pool(name="zb", bufs=2, space="PSUM") as zbp:

            M0_s = cst.tile([128, (NM + 1) * 128], f16)
            fWhT_s = cst.tile([WID, 3 * WID], f16)
            fWoT_s = cst.tile([WID, NCHUNK * 128], f16)
            b0_s = cst.tile([WID, 1], f32)
            bh_s = cst.tile([WID, 3], f32)
            B_s = cst.tile([128, BL], f16)       # base carry W0 @ y_n
            D2_s = cst.tile([128, BL], f16)
            Frep_s = cst.tile([128, ZF], f32)

            nc.sync.dma_start(M0_s[:, :], d_M0.ap()[:, :])
            nc.sync.dma_start(fWhT_s[:, :], d_fWhT.ap()[:, :])
            nc.sync.dma_start(fWoT_s[:, :], d_fWoT.ap()[:, :])
            nc.sync.dma_start(b0_s[:, :], d_b0.ap()[:, :])
            nc.sync.dma_start(bh_s[:, :], d_bh.ap()[:, :])
            nc.sync.dma_start(B_s[:, :], d_B0.ap()[:, :])
            if use_frep:
                nc.sync.dma_start(Frep_s[:, :], d_Frep.ap()[:, :])

            warm = cst.tile([1, 1], f32)
            nc.scalar.activation(warm[:, :], b0_s[0:1, 0:1], AF.Exp)
            nc.scalar.activation(warm[:, :], warm[:, :], AF.Ln, bias=1.0)

            xr_flat = d_xr.ap()

            # per-slot persistent head-group + merged rq tiles (fp16)
            rgA = [rqp.tile([128, BL], f16, tag=f"rgA{s}", name=f"rgA{s}")
                   for s in range(NST)]
            rgB = [rqp.tile([128, BL], f16, tag=f"rgB{s}", name=f"rgB{s}")
                   for s in range(NST)]
            mrg = [rqp.tile([128, BL], f16, tag=f"mrg{s}", name=f"mrg{s}")
                   for s in range(NST)]
            Dt = [rqp.tile([128, BL], f16, tag=f"D{s}", name=f"D{s}")
                  for s in range(NST - 1)]   # D for stages 3..7
            slotread = [xrp.tile([128, XF], f16, tag=f"xrs{s}", name=f"xrs{s}")
                        for s in range(NST)]

            def eval_chain(u1ps, xrt, rga, rgb, merged):
                """u1ps: assembled PSUM [128, BL]. Emits MLP + head; writes
                group reductions into rga/rgb and their sum into merged."""
                e0 = hp.tile([WID, BL], f32, tag="e", bufs=2)
                nc.scalar.activation(e0[:, :], u1ps[:, :], AF.Exp, bias=b0_s[:, 0:1])
                h = hp.tile([WID, BL], f16, tag="h", bufs=3)
                nc.scalar.activation(h[:, :], e0[:, :], AF.Ln, bias=1.0)
                for l in range(3):
                    pl = layp.tile([WID, BL], f32, tag="lay")
                    nc.tensor.matmul(pl[:, :], fWhT_s[:, 128 * l:128 * (l + 1)],
                                     h[:, :], start=True, stop=True)
                    el = hp.tile([WID, BL], f32, tag="e", bufs=2)
                    nc.scalar.activation(el[:, :], pl[:, :], AF.Exp,
                                         bias=bh_s[:, l:l + 1])
                    h = hp.tile([WID, BL], f16, tag="h", bufs=3)
                    nc.scalar.activation(h[:, :], el[:, :], AF.Ln, bias=1.0)

                zA = zap.tile([128, ZA], f32, tag="za")
                zB = zbp.tile([128, ZF - ZA], f32, tag="zb")
                for q in range(NQA):
                    nc.tensor.matmul(zA[:, 8 * q:8 * (q + 1)],
                                     fWoT_s[:, 128 * q:128 * (q + 1)],
                                     h[:, :], start=True, stop=True,
                                     skip_group_check=True)
                for q in range(NQB):
                    qq = NQA + q
                    nc.tensor.matmul(zB[:, 8 * q:8 * (q + 1)],
                                     fWoT_s[:, 128 * qq:128 * (qq + 1)],
                                     h[:, :], start=True, stop=True,
                                     skip_group_check=True)

                for g, (zps, cols, nq, rg) in enumerate(
                        [(zA, slice(0, ZA), NQA, rga),
                         (zB, slice(ZA, ZF), NQB, rgb)]):
                    w = nq * 8
                    E = bigp.tile([128, w], f32, tag=f"E{g}", name=f"E{g}")
                    nc.scalar.activation(E[:, :], zps[:, :], AF.Exp, scale=2.0)
                    dd = ddp.tile([128, w], f32, tag=f"dd{g}", name=f"dd{g}")
                    if use_frep:
                        nc.vector.tensor_tensor(dd[:, :], E[:, :],
                                                Frep_s[:, cols], op=ALU.mult)
                        nc.vector.tensor_scalar(dd[:, :], dd[:, :], 1.0, 1e30,
                                                op0=ALU.add, op1=ALU.min)
                    else:
                        nc.vector.tensor_scalar(dd[:, :], E[:, :], 1.0, 1e30,
                                                op0=ALU.add, op1=ALU.min)
                    rr = ddp.tile([128, w], f32, tag=f"rr{g}", name=f"rr{g}")
                    nc.vector.reciprocal_approx_fast(rr[:, :], dd[:, :])
                    qd = bigp.tile([128, w], f32, tag=f"qd{g}", name=f"qd{g}")
                    nc.vector.tensor_tensor(qd[:, :], xrt[:, cols], rr[:, :],
                                            op=ALU.mult)
                    nc.vector.tensor_reduce(
                        rg[:, :],
                        qd[:, :].rearrange("p (q b) -> p b q", q=nq),
                        axis=mybir.AxisListType.X, op=ALU.add)
                nc.vector.tensor_tensor(merged[:, :], rga[:, :], rgb[:, :],
                                        op=ALU.add)

            def u1_batch(u1, j, Dj):
                """Off-chain part of u1_j assembly: id-MM + old-rq terms."""
                nc.tensor.matmul(u1[:, :], M0_s[:, id_col:id_col + 128],
                                 Dj[:, :], start=True, stop=False,
                                 skip_group_check=True)
                for m in range(1, j - 1):
                    if _A[j, m] == 0.0:
                        continue
                    col = m0_col[(j, m)]
                    rhs = mrg[NST - 1] if m == 1 else mrg[m - 2]
                    nc.tensor.matmul(u1[:, :], M0_s[:, col:col + 128],
                                     rhs[:, :], start=False, stop=False,
                                     skip_group_check=True)

            def u1_chain(u1, j, prev_a, prev_b):
                """Chain head: last M0a term consuming the two group tiles."""
                col = m0_col[(j, j - 1)]
                nc.tensor.matmul(u1[:, :], M0_s[:, col:col + 128],
                                 prev_a[:, :], start=False, stop=False,
                                 skip_group_check=True)
                nc.tensor.matmul(u1[:, :], M0_s[:, col:col + 128],
                                 prev_b[:, :], start=False, stop=True,
                                 skip_group_check=True)

            # ---------------- pre-loop ----------------
            xr0 = xrp.tile([128, XF], f16, tag="xr0")
            nc.sync.dma_start(xr0[:, :], xr_flat[bass.DynSlice(0, 1), :, :])
            u1i = u1p.tile([128, BL], f32, tag="u1")
            nc.tensor.matmul(u1i[:, :], M0_s[:, id_col:id_col + 128], B_s[:, :],
                             start=True, stop=True, skip_group_check=True)
            # initial eval writes the slot-5 tiles (rq_1 for interval 0)
            eval_chain(u1i, xr0, rgA[NST - 1], rgB[NST - 1], mrg[NST - 1])
            nc.sync.dma_start(d_rq.ap()[bass.DynSlice(0, 1), :, :],
                              mrg[NST - 1][:, :])

            nc.sync.dma_start(slotread[0][:, :], xr_flat[bass.DynSlice(1, 1), :, :])
            nc.sync.dma_start(slotread[1][:, :], xr_flat[bass.DynSlice(2, 1), :, :])

            # D_2 and u1_2 batch for interval 0
            nc.vector.tensor_tensor(D2_s[:, :], B_s[:, :], slotread[0][:, ZF:XF],
                                    op=ALU.add)
            u1_first = u1p.tile([128, BL], f32, tag="u1")
            u1_batch(u1_first, 2, D2_s)

            u1_tiles = {2: u1_first}

            hints = (mybir.EngineType.PE, mybir.EngineType.Activation,
                     mybir.EngineType.DVE, mybir.EngineType.SP)
            with tc.For_i(0, NI, 1, hint_engines=hints,
                          staggered_reset=STAGGERED) as iv:
                for s in range(NST):
                    j = s + 2

                    # ---- chain: finish u1_j and run the eval
                    u1 = u1_tiles.pop(j)
                    pa = rgA[NST - 1] if s == 0 else rgA[s - 1]
                    pb = rgB[NST - 1] if s == 0 else rgB[s - 1]
                    u1_chain(u1, j, pa, pb)
                    eval_chain(u1, slotread[s], rgA[s], rgB[s], mrg[s])

                    nc.sync.dma_start(
                        d_rq.ap()[bass.DynSlice(iv * NST + (s + 1), 1), :, :],
                        mrg[s][:, :])

                    nc.sync.dma_start(
                        slotread[(s + 2) % NST][:, :],
                        xr_flat[bass.DynSlice(iv * NST + (s + 3), 1), :, :])

                    # ---- off-chain: D + u1 batch for stage j+1 (or next
                    # interval's stage 2 at s == 5)
                    if s < NST - 1:
                        nc.vector.tensor_tensor(Dt[s][:, :], B_s[:, :],
                                                slotread[s + 1][:, ZF:XF],
                                                op=ALU.add)
                        u1n = u1p.tile([128, BL], f32, tag="u1", name=f"u1n{s}")
                        u1_batch(u1n, j + 1, Dt[s])
                        u1_tiles[j + 1] = u1n
                    else:
                        # B_{n+1} = u1_7 content (stage-7 state = y_{n+1})
                        nc.vector.tensor_copy(B_s[:, :], u1[:, :])
                        nc.vector.tensor_tensor(D2_s[:, :], B_s[:, :],
                                                slotread[0][:, ZF:XF],
                                                op=ALU.add)
                        u1n = u1p.tile([128, BL], f32, tag="u1", name="u1n5")
                        u1_batch(u1n, 2, D2_s)
                        u1_tiles[2] = u1n

    nc.compile()
    return nc


# ----------------------------------------------------------------- interface

def kernel(ts, ys, iW0, ib0, iWh, ibh, iWo, ibo, fW0, fb0, fWh, fbh, fWo, fbo,
           lW, lb):
    from concourse import bass_utils

    f32 = np.float32
    to_np = lambda a: np.asarray(a, dtype=f32)
    ts, ys = to_np(ts), to_np(ys)
    iW0, ib0, iWh, ibh = to_np(iW0), to_np(ib0), to_np(iWh), to_np(ibh)
    iWo, ibo = to_np(iWo), to_np(ibo)
    fW0, fb0, fWh, fbh = to_np(fW0), to_np(fb0), to_np(fWh), to_np(fbh)
    fWo, fbo, lW, lb = to_np(fWo), to_np(fbo), to_np(lW), to_np(lb)

    xr_cores, M0cat, B0, y0, S_all, S0 = _host_precompute(
        ts, ys, iW0, ib0, iWh, ibh, iWo, ibo, fW0, fWo)

    # c-major permuted fWo (baseline layout) + Frep fallback
    perm = np.zeros(CP * HID, np.int64) - 1
    csrc = np.arange(C)
    for h_i in range(HID):
        perm[csrc * HID + h_i] = h_i * C + csrc
    fWo_cm = np.zeros((CP * HID, WID), f32)
    fbo_cm = np.zeros((CP * HID,), f32)
    valid = perm >= 0
    fWo_cm[valid] = fWo[perm[valid]]
    fbo_cm[valid] = fbo[perm[valid]]
    fWoT = np.ascontiguousarray(
        np.concatenate([fWo_cm[128 * q:128 * (q + 1)].T for q in range(NCHUNK)],
                       axis=1)).astype(np.float16)
    Frep = np.exp(2.0 * fbo_cm.reshape(NCHUNK, 128)).T
    Frep = np.repeat(Frep[:, :, None], 8, axis=2).reshape(128, ZF).astype(f32)

    use_frep = bool(np.any(fbo))
    global _COMPILED
    if _COMPILED is None or _COMPILED[0] != use_frep:
        _COMPILED = (use_frep, _build(use_frep=use_frep))
    nc = _COMPILED[1]

    fWhT = np.ascontiguousarray(
        np.concatenate([fWh[k].T for k in range(3)], axis=1)).astype(np.float16)

    in_maps = []
    for core in range(N_CORES):
        in_maps.append({
            "xr": xr_cores[core],
            "M0cat": M0cat,
            "B0": B0[core],
            "fWhT": fWhT,
            "fWoT": fWoT,
            "fb0c": fb0[:, None],
            "fbhc": np.ascontiguousarray(fbh.T),
            "Frep": Frep,
        })

    global _LAST_IN_MAPS
    _LAST_IN_MAPS = in_maps
    res = bass_utils.run_bass_kernel_spmd(nc, in_maps, core_ids=list(range(N_CORES)))

    rq_cores = [res.results[core]["rq"] for core in range(N_CORES)]
    ysol = _host_reconstruct(rq_cores, y0, S_all, S0, lW, lb)
    out = ysol @ lW.T + lb[None, None, :]
    return out.astype(f32)


if __name__ == "__main__":
    pass


# revision 7
# speedup vs baseline: 2.9687x; 1.0349x over previous
"""NeuralCDE forward on 8 Trainium2 NeuronCores — v2.

The reference integrates with RK4 x 4 substeps (16 MLP evals/interval).
The wall-clock is bound by the *serial* eval chain (batch width is nearly
free), so v2:

1. Integrates with DOPRI5 + FSAL: 6 evals/interval (vs 16), validated
   rel_err ~1.8e-3 vs the reference (gate 2e-2).
2. Shortens each eval's chain by linearity-folding the stage combines:
   the stage state y_j is never materialized on-chain. The first-layer
   pre-activation u1_j = W0 @ y_j decomposes as
     u1_j = B_n + xsA_j + sum_m (-2 a_jm) * (W0.fold) @ rq_m
   where rq_m is the per-eval head reduction, B_n = W0 @ y_n carries via
   B_{n+1} = u1_7 (dopri5's 7th stage state IS y_{n+1}), and xsA_j is a
   host-precomputed rank-1 term. All combines are PE matmuls with
   pre-scaled stationaries (M0a); the old S-matmul/DVE stage tail is gone.
3. Streams rq out; the host reconstructs y (K_m = S_m - 2*fold(rq_m)) and
   applies the readout. fbo == 0 assumed (checked; Frep fallback built
   on demand as in the baseline).
4. Head is split into two chunk groups (9+8) so Exp/DVE overlap the
   chunk matmuls; softplus stays Exp+Ln on ACT (single act-table set).
"""

import numpy as np

N_CORES = 8
T = 128
B = 64
OBS = 32
HID = 64
WID = 128
OUT = 32
C = OBS + 1          # 33
CP = 34              # padded C (even)
NCHUNK = 17          # 2176 / 128
ZF = NCHUNK * 8      # 136
XF = ZF + 8          # 144: xrep ++ xsA
NI = T - 1           # 127 intervals
NST = 6              # dopri5 evals per interval (stages 2..7)
NEV = 1 + NI * NST   # total evals incl. initial k1
BL = B // N_CORES    # 8 per core
NQA = 9              # chunks in head group A
NQB = NCHUNK - NQA   # 8
ZA = NQA * 8         # 72
STAGGERED = True

_COMPILED = None
_LAST_IN_MAPS = None

# dopri5 tableau (row 7 = b; FSAL)
_A = np.zeros((8, 8))
_A[2, 1] = 1 / 5
_A[3, 1:3] = [3 / 40, 9 / 40]
_A[4, 1:4] = [44 / 45, -56 / 15, 32 / 9]
_A[5, 1:5] = [19372 / 6561, -25360 / 2187, 64448 / 6561, -212 / 729]
_A[6, 1:6] = [9017 / 3168, -355 / 33, 46732 / 5247, 49 / 176, -5103 / 18656]
_A[7, 1:7] = [35 / 384, 0.0, 500 / 1113, 125 / 192, -2187 / 6784, 11 / 84]
_CS = [0.0, 0.0, 1 / 5, 3 / 10, 4 / 5, 8 / 9, 1.0, 1.0]
# (j, m) pairs with a_jm != 0, in emission order per stage
_JM = [(j, m) for j in range(2, 8) for m in range(1, j) if _A[j, m] != 0.0]
NM = len(_JM)        # 20


# ----------------------------------------------------------------- host math

def _host_precompute(ts, ys, iW0, ib0, iWh, ibh, iWo, ibo, fW0, fWo):
    f32 = np.float32
    ts = ts.astype(f32)
    ys = ys.astype(f32)

    tys = np.concatenate([np.broadcast_to(ts[None, :, None], (B, T, 1)), ys], axis=-1)
    dts = ts[1:] - ts[:-1]
    diffs = (tys[:, 1:] - tys[:, :-1]) / dts[None, :, None]
    deriv = np.concatenate([diffs[:, :1], diffs], axis=1)
    d0 = deriv[:, :-1]
    d1 = deriv[:, 1:]
    cc = (3.0 * diffs - 2.0 * d0 - d1) / dts[None, :, None]
    bb = (d0 + d1 - 2.0 * diffs) / (dts * dts)[None, :, None]

    # X[b, i, jj, c] = h * xdot at stage (jj+2)'s c-point; X0 = initial c=0
    cpts = np.array([_CS[j] for j in range(2, 8)], f32)
    s = (cpts[None, :] * dts[:, None])[None, :, :, None]
    X = (d0[:, :, None, :] + 2.0 * cc[:, :, None, :] * s
         + 3.0 * bb[:, :, None, :] * s * s) * dts[None, :, None, None]
    X = X.astype(f32)                                  # (B, NI, 6, C)
    X0 = (d0[:, 0] * dts[0]).astype(f32)               # (B, C)

    S_all = X.sum(-1)                                  # (B, NI, 6)
    S0 = X0.sum(-1)                                    # (B,)
    rowsumW0 = fW0.sum(axis=1).astype(f32)             # (128,)

    # y0 via init MLP
    relu = lambda v: np.maximum(v, 0.0, dtype=f32)
    h = relu(tys[:, 0] @ iW0.T + ib0[None, :])
    for k in range(iWh.shape[0]):
        h = relu(h @ iWh[k].T + ibh[k][None, :])
    y0 = (h @ iWo.T + ibo[None, :]).astype(f32)        # (B, HID)

    # xsA scalars per (i, jj): sum_{m<j} a_jm * S_m  -> (B, NI, 6)
    xsA_s = np.zeros((B, NI, NST), f32)
    for jj in range(NST):
        j = jj + 2
        for m in range(1, j):
            a = _A[j, m]
            if a == 0.0:
                continue
            if m == 1:
                Sm = np.concatenate([S0[:, None], S_all[:, :-1, 5]], axis=1)  # (B, NI)
            else:
                Sm = S_all[:, :, m - 2]
            xsA_s[:, :, jj] += np.float32(a) * Sm

    # per-core xr tiles: [NEV(+pad), 128, XF]: xrep cols 0:136, xsA cols 136:144
    q_idx = np.arange(NCHUNK)
    part_half = np.arange(128) // 64
    cmap = (2 * q_idx[None, :] + part_half[:, None])   # (128, 17)

    xr_cores = []
    for core in range(N_CORES):
        sl = slice(core * BL, (core + 1) * BL)
        Xp = np.zeros((BL, NI, NST, CP), f32)
        Xp[..., :C] = X[sl]
        xr = Xp[:, :, :, cmap]                         # (BL, NI, 6, 128, 17)
        xr = xr.transpose(1, 2, 3, 4, 0).reshape(NI * NST, 128, ZF)
        xsA = rowsumW0[None, :, None] * xsA_s[sl].transpose(1, 2, 0).reshape(
            NI * NST, 1, BL)                           # (NI*6, 128, BL)
        tiles = np.zeros((NEV + NST, 128, XF), np.float16)  # +NST zero-pad
        tiles[1:NEV, :, :ZF] = xr
        tiles[1:NEV, :, ZF:] = xsA
        X0p = np.zeros((BL, CP), f32)
        X0p[:, :C] = X0[sl]
        tiles[0, :, :ZF] = X0p[:, cmap].transpose(1, 2, 0).reshape(128, ZF)
        xr_cores.append(np.ascontiguousarray(tiles))

    # M0 stationaries (lhsT layout): M0a_{jm} = (-2 a_jm) * W0F, W0F[o,p]=W0[o,p%64]
    W0F = np.concatenate([fW0, fW0], axis=1).astype(f32)      # (128, 128)
    mats = [np.ascontiguousarray((np.float32(-2.0 * _A[j, m]) * W0F).T)
            for (j, m) in _JM]
    mats.append(np.eye(128, dtype=f32))                       # identity last
    M0cat = np.concatenate(mats, axis=1).astype(np.float16)   # (128, (NM+1)*128)

    B0 = np.stack([np.ascontiguousarray(fW0 @ y0[c * BL:(c + 1) * BL].T)
                   for c in range(N_CORES)]).astype(np.float16)

    return xr_cores, M0cat, B0, y0, S_all, S0


def _host_reconstruct(rq_cores, y0, S_all, S0, lW, lb):
    f32 = np.float32
    # stack cores on batch axis: rq_full [NEV, 128, B]
    rq = np.concatenate([rq_cores[c] for c in range(N_CORES)], axis=2).astype(f32)
    K = -2.0 * (rq[:, :HID, :] + rq[:, HID:, :]).transpose(0, 2, 1)  # (NEV, B, HID)
    # add S_m per eval
    K[0] += S0[:, None]
    K[1:] += S_all.reshape(B, NI * NST).T[:, :, None]
    bvec = _A[7]
    ysol = np.zeros((B, T, HID), f32)
    ysol[:, 0] = y0
    y = y0.copy()
    for i in range(NI):
        k1 = K[0] if i == 0 else K[1 + (i - 1) * NST + 5]
        dy = np.float32(bvec[1]) * k1
        for m in range(3, 8):        # b2 == 0
            dy += np.float32(bvec[m]) * K[1 + i * NST + (m - 2)]
        y = y + dy
        ysol[:, i + 1] = y
    return ysol


# ------------------------------------------------------------- device kernel

def _patch_act_tables():
    """Keep Exp/Ln only in their shared table set so a single
    ACT_TABLE_LOAD is hoisted (see baseline)."""
    import concourse.bacc as bacc
    import concourse.hw_specs as hw_specs
    import concourse.mybir as mybir

    if getattr(bacc, "_act_tables_patched", False):
        return
    Tt = mybir.ActivationFunctionType
    orig = hw_specs.get_activation_tables

    def patched(arch):
        tabs = orig(arch)
        for name, s in tabs.items():
            if name != "natural_log_exp_and_others":
                s.discard(Tt.Exp)
                s.discard(Tt.Ln)
        return tabs

    bacc.get_activation_tables = patched
    bacc._act_tables_patched = True


def _build(use_frep=False):
    import concourse.bass as bass
    import concourse.bacc as bacc
    import concourse.mybir as mybir
    import concourse.tile as tile

    _patch_act_tables()
    AF = mybir.ActivationFunctionType
    ALU = mybir.AluOpType
    f32 = mybir.dt.float32
    f16 = mybir.dt.float16

    nc = bacc.Bacc("TRN2", num_devices=N_CORES)

    d_xr = nc.dram_tensor("xr", [NEV + NST, 128, XF], f16, kind="ExternalInput")
    d_M0 = nc.dram_tensor("M0cat", [128, (NM + 1) * 128], f16, kind="ExternalInput")
    d_B0 = nc.dram_tensor("B0", [128, BL], f16, kind="ExternalInput")
    d_fWhT = nc.dram_tensor("fWhT", [WID, 3 * WID], f16, kind="ExternalInput")
    d_fWoT = nc.dram_tensor("fWoT", [WID, NCHUNK * 128], f16, kind="ExternalInput")
    d_b0 = nc.dram_tensor("fb0c", [WID, 1], f32, kind="ExternalInput")
    d_bh = nc.dram_tensor("fbhc", [WID, 3], f32, kind="ExternalInput")
    d_Frep = nc.dram_tensor("Frep", [128, ZF], f32, kind="ExternalInput")
    d_rq = nc.dram_tensor("rq", [NEV, 128, BL], f16, kind="ExternalOutput")

    m0_col = {jm: 128 * k for k, jm in enumerate(_JM)}
    id_col = 128 * NM

    with tile.TileContext(nc) as tc, \
         nc.allow_low_precision("fp16 rq stream validated offline (4.7e-3)"):
        with tc.tile_pool(name="const", bufs=1) as cst, \
             tc.tile_pool(name="xr", bufs=1) as xrp, \
             tc.tile_pool(name="h", bufs=2) as hp, \
             tc.tile_pool(name="big", bufs=2) as bigp, \
             tc.tile_pool(name="rqs", bufs=1) as rqp, \
             tc.tile_pool(name="dd", bufs=2) as ddp, \
             tc.tile_pool(name="u1", bufs=2, space="PSUM") as u1p, \
             tc.tile_pool(name="lay", bufs=2, space="PSUM") as layp, \
             tc.tile_pool(name="ep", bufs=2, space="PSUM") as epp, \
             tc.tile_pool(name="z", bufs=2, space="PSUM") as zap:

            M0_s = cst.tile([128, (NM + 1) * 128], f16)
            fWhT_s = cst.tile([WID, 3 * WID], f16)
            fWoT_s = cst.tile([WID, NCHUNK * 128], f16)
            b0_s = cst.tile([WID, 1], f32)
            bh_s = cst.tile([WID, 3], f32)
            B_s = cst.tile([128, BL], f16)       # base carry W0 @ y_n
            D2_s = cst.tile([128, BL], f16)
            Frep_s = cst.tile([128, ZF], f32)

            nc.sync.dma_start(M0_s[:, :], d_M0.ap()[:, :])
            nc.sync.dma_start(fWhT_s[:, :], d_fWhT.ap()[:, :])
            nc.sync.dma_start(fWoT_s[:, :], d_fWoT.ap()[:, :])
            nc.sync.dma_start(b0_s[:, :], d_b0.ap()[:, :])
            nc.sync.dma_start(bh_s[:, :], d_bh.ap()[:, :])
            nc.sync.dma_start(B_s[:, :], d_B0.ap()[:, :])
            if use_frep:
                nc.sync.dma_start(Frep_s[:, :], d_Frep.ap()[:, :])

            warm = cst.tile([1, 1], f32)
            nc.scalar.activation(warm[:, :], b0_s[0:1, 0:1], AF.Exp)
            nc.scalar.activation(warm[:, :], warm[:, :], AF.Ln, bias=1.0)

            xr_flat = d_xr.ap()

            # per-slot merged rq tiles (fp16)
            mrg = [rqp.tile([128, BL], f16, tag=f"mrg{s}", name=f"mrg{s}")
                   for s in range(NST)]
            Dt = [rqp.tile([128, BL], f16, tag=f"D{s}", name=f"D{s}")
                  for s in range(NST - 1)]   # D for stages 3..7
            slotread = [xrp.tile([128, XF], f16, tag=f"xrs{s}", name=f"xrs{s}")
                        for s in range(NST)]

            def eval_chain(u1ps, xrt, merged):
                """u1ps: assembled PSUM [128, BL]. Emits MLP + head; writes
                the head reduction into merged [128, BL] fp16."""
                e0 = epp.tile([WID, BL], f32, tag="e")
                nc.scalar.activation(e0[:, :], u1ps[:, :], AF.Exp, bias=b0_s[:, 0:1])
                h = hp.tile([WID, BL], f16, tag="h", bufs=3)
                nc.scalar.activation(h[:, :], e0[:, :], AF.Ln, bias=1.0)
                for l in range(3):
                    pl = layp.tile([WID, BL], f32, tag="lay")
                    nc.tensor.matmul(pl[:, :], fWhT_s[:, 128 * l:128 * (l + 1)],
                                     h[:, :], start=True, stop=True)
                    el = epp.tile([WID, BL], f32, tag="e")
                    nc.scalar.activation(el[:, :], pl[:, :], AF.Exp,
                                         bias=bh_s[:, l:l + 1])
                    h = hp.tile([WID, BL], f16, tag="h", bufs=3)
                    nc.scalar.activation(h[:, :], el[:, :], AF.Ln, bias=1.0)

                zps = zap.tile([128, ZF], f32, tag="z")
                for q in range(NCHUNK):
                    nc.tensor.matmul(zps[:, 8 * q:8 * (q + 1)],
                                     fWoT_s[:, 128 * q:128 * (q + 1)],
                                     h[:, :], start=True, stop=True,
                                     skip_group_check=True)

                E = bigp.tile([128, ZF], f32, tag="E")
                nc.scalar.activation(E[:, :], zps[:, :], AF.Exp, scale=2.0)
                dd = ddp.tile([128, ZF], f32, tag="dd")
                if use_frep:
                    nc.vector.tensor_tensor(dd[:, :], E[:, :], Frep_s[:, :],
                                            op=ALU.mult)
                    nc.vector.tensor_scalar(dd[:, :], dd[:, :], 1.0, 1e30,
                                            op0=ALU.add, op1=ALU.min)
                else:
                    nc.vector.tensor_scalar(dd[:, :], E[:, :], 1.0, 1e30,
                                            op0=ALU.add, op1=ALU.min)
                rr = ddp.tile([128, ZF], f32, tag="rr")
                nc.vector.reciprocal_approx_fast(rr[:, :], dd[:, :])
                qd = bigp.tile([128, ZF], f32, tag="qd")
                nc.vector.tensor_tensor(qd[:, :], xrt[:, 0:ZF], rr[:, :],
                                        op=ALU.mult)
                nc.vector.tensor_reduce(
                    merged[:, :],
                    qd[:, :].rearrange("p (q b) -> p b q", q=NCHUNK),
                    axis=mybir.AxisListType.X, op=ALU.add)

            def u1_batch(u1, j, Dj):
                """Off-chain part of u1_j assembly: id-MM + old-rq terms."""
                nc.tensor.matmul(u1[:, :], M0_s[:, id_col:id_col + 128],
                                 Dj[:, :], start=True, stop=False,
                                 skip_group_check=True)
                for m in range(1, j - 1):
                    if _A[j, m] == 0.0:
                        continue
                    col = m0_col[(j, m)]
                    rhs = mrg[NST - 1] if m == 1 else mrg[m - 2]
                    nc.tensor.matmul(u1[:, :], M0_s[:, col:col + 128],
                                     rhs[:, :], start=False, stop=False,
                                     skip_group_check=True)

            def u1_chain(u1, j, prev):
                """Chain head: last M0a term consuming the previous rq."""
                col = m0_col[(j, j - 1)]
                nc.tensor.matmul(u1[:, :], M0_s[:, col:col + 128],
                                 prev[:, :], start=False, stop=True,
                                 skip_group_check=True)

            # ---------------- pre-loop ----------------
            xr0 = xrp.tile([128, XF], f16, tag="xr0")
            nc.sync.dma_start(xr0[:, :], xr_flat[bass.DynSlice(0, 1), :, :])
            u1i = u1p.tile([128, BL], f32, tag="u1")
            nc.tensor.matmul(u1i[:, :], M0_s[:, id_col:id_col + 128], B_s[:, :],
                             start=True, stop=True, skip_group_check=True)
            # initial eval writes the slot-5 tile (rq_1 for interval 0)
            eval_chain(u1i, xr0, mrg[NST - 1])
            nc.sync.dma_start(d_rq.ap()[bass.DynSlice(0, 1), :, :],
                              mrg[NST - 1][:, :])

            nc.sync.dma_start(slotread[0][:, :], xr_flat[bass.DynSlice(1, 1), :, :])
            nc.sync.dma_start(slotread[1][:, :], xr_flat[bass.DynSlice(2, 1), :, :])

            # D_2 and u1_2 batch for interval 0
            nc.vector.tensor_tensor(D2_s[:, :], B_s[:, :], slotread[0][:, ZF:XF],
                                    op=ALU.add)
            u1_first = u1p.tile([128, BL], f32, tag="u1")
            u1_batch(u1_first, 2, D2_s)

            u1_tiles = {2: u1_first}

            hints = (mybir.EngineType.PE, mybir.EngineType.Activation,
                     mybir.EngineType.DVE, mybir.EngineType.SP)
            with tc.For_i(0, NI, 1, hint_engines=hints,
                          staggered_reset=STAGGERED) as iv:
                for s in range(NST):
                    j = s + 2

                    # ---- chain: finish u1_j and run the eval
                    u1 = u1_tiles.pop(j)
                    prev = mrg[NST - 1] if s == 0 else mrg[s - 1]
                    u1_chain(u1, j, prev)
                    eval_chain(u1, slotread[s], mrg[s])

                    nc.sync.dma_start(
                        d_rq.ap()[bass.DynSlice(iv * NST + (s + 1), 1), :, :],
                        mrg[s][:, :])

                    nc.sync.dma_start(
                        slotread[(s + 2) % NST][:, :],
                        xr_flat[bass.DynSlice(iv * NST + (s + 3), 1), :, :])

                    # ---- off-chain: D + u1 batch for stage j+1 (or next
                    # interval's stage 2 at s == 5)
                    if s < NST - 1:
                        nc.vector.tensor_tensor(Dt[s][:, :], B_s[:, :],
                                                slotread[s + 1][:, ZF:XF],
                                                op=ALU.add)
                        u1n = u1p.tile([128, BL], f32, tag="u1", name=f"u1n{s}")
                        u1_batch(u1n, j + 1, Dt[s])
                        u1_tiles[j + 1] = u1n
                    else:
                        # B_{n+1} = u1_7 content (stage-7 state = y_{n+1})
                        nc.vector.tensor_copy(B_s[:, :], u1[:, :])
                        nc.vector.tensor_tensor(D2_s[:, :], B_s[:, :],
                                                slotread[0][:, ZF:XF],
                                                op=ALU.add)
                        u1n = u1p.tile([128, BL], f32, tag="u1", name="u1n5")
                        u1_batch(u1n, 2, D2_s)
                        u1_tiles[2] = u1n

    nc.compile()
    return nc


# ----------------------------------------------------------------- interface

def kernel(ts, ys, iW0, ib0, iWh, ibh, iWo, ibo, fW0, fb0, fWh, fbh, fWo, fbo,
           lW, lb):
    from concourse import bass_utils

    f32 = np.float32
    to_np = lambda a: np.asarray(a, dtype=f32)
    ts, ys = to_np(ts), to_np(ys)
    iW0, ib0, iWh, ibh = to_np(iW0), to_np(ib0), to_np(iWh), to_np(ibh)
    iWo, ibo = to_np(iWo), to_np(ibo)
    fW0, fb0, fWh, fbh = to_np(fW0), to_np(fb0), to_np(fWh), to_np(fbh)
    fWo, fbo, lW, lb = to_np(fWo), to_np(fbo), to_np(lW), to_np(lb)

    xr_cores, M0cat, B0, y0, S_all, S0 = _host_precompute(
        ts, ys, iW0, ib0, iWh, ibh, iWo, ibo, fW0, fWo)

    # c-major permuted fWo (baseline layout) + Frep fallback
    perm = np.zeros(CP * HID, np.int64) - 1
    csrc = np.arange(C)
    for h_i in range(HID):
        perm[csrc * HID + h_i] = h_i * C + csrc
    fWo_cm = np.zeros((CP * HID, WID), f32)
    fbo_cm = np.zeros((CP * HID,), f32)
    valid = perm >= 0
    fWo_cm[valid] = fWo[perm[valid]]
    fbo_cm[valid] = fbo[perm[valid]]
    fWoT = np.ascontiguousarray(
        np.concatenate([fWo_cm[128 * q:128 * (q + 1)].T for q in range(NCHUNK)],
                       axis=1)).astype(np.float16)
    Frep = np.exp(2.0 * fbo_cm.reshape(NCHUNK, 128)).T
    Frep = np.repeat(Frep[:, :, None], 8, axis=2).reshape(128, ZF).astype(f32)

    use_frep = bool(np.any(fbo))
    global _COMPILED
    if _COMPILED is None or _COMPILED[0] != use_frep:
        _COMPILED = (use_frep, _build(use_frep=use_frep))
    nc = _COMPILED[1]

    fWhT = np.ascontiguousarray(
        np.concatenate([fWh[k].T for k in range(3)], axis=1)).astype(np.float16)

    in_maps = []
    for core in range(N_CORES):
        in_maps.append({
            "xr": xr_cores[core],
            "M0cat": M0cat,
            "B0": B0[core],
            "fWhT": fWhT,
            "fWoT": fWoT,
            "fb0c": fb0[:, None],
            "fbhc": np.ascontiguousarray(fbh.T),
            "Frep": Frep,
        })

    global _LAST_IN_MAPS
    _LAST_IN_MAPS = in_maps
    res = bass_utils.run_bass_kernel_spmd(nc, in_maps, core_ids=list(range(N_CORES)))

    rq_cores = [res.results[core]["rq"] for core in range(N_CORES)]
    ysol = _host_reconstruct(rq_cores, y0, S_all, S0, lW, lb)
    out = ysol @ lW.T + lb[None, None, :]
    return out.astype(f32)


if __name__ == "__main__":
    pass


# revision 8
# speedup vs baseline: 2.9774x; 1.0029x over previous
"""NeuralCDE forward on 8 Trainium2 NeuronCores — v2.

The reference integrates with RK4 x 4 substeps (16 MLP evals/interval).
The wall-clock is bound by the *serial* eval chain (batch width is nearly
free), so v2:

1. Integrates with DOPRI5 + FSAL: 6 evals/interval (vs 16), validated
   rel_err ~1.8e-3 vs the reference (gate 2e-2).
2. Shortens each eval's chain by linearity-folding the stage combines:
   the stage state y_j is never materialized on-chain. The first-layer
   pre-activation u1_j = W0 @ y_j decomposes as
     u1_j = B_n + xsA_j + sum_m (-2 a_jm) * (W0.fold) @ rq_m
   where rq_m is the per-eval head reduction, B_n = W0 @ y_n carries via
   B_{n+1} = u1_7 (dopri5's 7th stage state IS y_{n+1}), and xsA_j is a
   host-precomputed rank-1 term. All combines are PE matmuls with
   pre-scaled stationaries (M0a); the old S-matmul/DVE stage tail is gone.
3. Streams rq out; the host reconstructs y (K_m = S_m - 2*fold(rq_m)) and
   applies the readout. fbo == 0 assumed (checked; Frep fallback built
   on demand as in the baseline).
4. Head is split into two chunk groups (9+8) so Exp/DVE overlap the
   chunk matmuls; softplus stays Exp+Ln on ACT (single act-table set).
"""

import numpy as np

N_CORES = 8
T = 128
B = 64
OBS = 32
HID = 64
WID = 128
OUT = 32
C = OBS + 1          # 33
CP = 34              # padded C (even)
NCHUNK = 17          # 2176 / 128
ZF = NCHUNK * 8      # 136
XF = ZF + 8          # 144: xrep ++ xsA
NI = T - 1           # 127 intervals
NST = 6              # dopri5 evals per interval (stages 2..7)
NEV = 1 + NI * NST   # total evals incl. initial k1
BL = B // N_CORES    # 8 per core
NQA = 9              # chunks in head group A
NQB = NCHUNK - NQA   # 8
ZA = NQA * 8         # 72
STAGGERED = True

_COMPILED = None
_LAST_IN_MAPS = None

# dopri5 tableau (row 7 = b; FSAL)
_A = np.zeros((8, 8))
_A[2, 1] = 1 / 5
_A[3, 1:3] = [3 / 40, 9 / 40]
_A[4, 1:4] = [44 / 45, -56 / 15, 32 / 9]
_A[5, 1:5] = [19372 / 6561, -25360 / 2187, 64448 / 6561, -212 / 729]
_A[6, 1:6] = [9017 / 3168, -355 / 33, 46732 / 5247, 49 / 176, -5103 / 18656]
_A[7, 1:7] = [35 / 384, 0.0, 500 / 1113, 125 / 192, -2187 / 6784, 11 / 84]
_CS = [0.0, 0.0, 1 / 5, 3 / 10, 4 / 5, 8 / 9, 1.0, 1.0]
# (j, m) pairs with a_jm != 0, in emission order per stage
_JM = [(j, m) for j in range(2, 8) for m in range(1, j) if _A[j, m] != 0.0]
NM = len(_JM)        # 20


# ----------------------------------------------------------------- host math

def _host_precompute(ts, ys, iW0, ib0, iWh, ibh, iWo, ibo, fW0, fWo):
    f32 = np.float32
    ts = ts.astype(f32)
    ys = ys.astype(f32)

    tys = np.concatenate([np.broadcast_to(ts[None, :, None], (B, T, 1)), ys], axis=-1)
    dts = ts[1:] - ts[:-1]
    diffs = (tys[:, 1:] - tys[:, :-1]) / dts[None, :, None]
    deriv = np.concatenate([diffs[:, :1], diffs], axis=1)
    d0 = deriv[:, :-1]
    d1 = deriv[:, 1:]
    cc = (3.0 * diffs - 2.0 * d0 - d1) / dts[None, :, None]
    bb = (d0 + d1 - 2.0 * diffs) / (dts * dts)[None, :, None]

    # X[b, i, jj, c] = h * xdot at stage (jj+2)'s c-point; X0 = initial c=0
    cpts = np.array([_CS[j] for j in range(2, 8)], f32)
    s = (cpts[None, :] * dts[:, None])[None, :, :, None]
    X = (d0[:, :, None, :] + 2.0 * cc[:, :, None, :] * s
         + 3.0 * bb[:, :, None, :] * s * s) * dts[None, :, None, None]
    X = X.astype(f32)                                  # (B, NI, 6, C)
    X0 = (d0[:, 0] * dts[0]).astype(f32)               # (B, C)

    S_all = X.sum(-1)                                  # (B, NI, 6)
    S0 = X0.sum(-1)                                    # (B,)
    rowsumW0 = fW0.sum(axis=1).astype(f32)             # (128,)

    # y0 via init MLP
    relu = lambda v: np.maximum(v, 0.0, dtype=f32)
    h = relu(tys[:, 0] @ iW0.T + ib0[None, :])
    for k in range(iWh.shape[0]):
        h = relu(h @ iWh[k].T + ibh[k][None, :])
    y0 = (h @ iWo.T + ibo[None, :]).astype(f32)        # (B, HID)

    # xsA scalars per (i, jj): sum_{m<j} a_jm * S_m  -> (B, NI, 6)
    xsA_s = np.zeros((B, NI, NST), f32)
    for jj in range(NST):
        j = jj + 2
        for m in range(1, j):
            a = _A[j, m]
            if a == 0.0:
                continue
            if m == 1:
                Sm = np.concatenate([S0[:, None], S_all[:, :-1, 5]], axis=1)  # (B, NI)
            else:
                Sm = S_all[:, :, m - 2]
            xsA_s[:, :, jj] += np.float32(a) * Sm

    # per-core xr tiles: [NEV(+pad), 128, XF]: xrep cols 0:136, xsA cols 136:144
    q_idx = np.arange(NCHUNK)
    part_half = np.arange(128) // 64
    cmap = (2 * q_idx[None, :] + part_half[:, None])   # (128, 17)

    xr_cores = []
    for core in range(N_CORES):
        sl = slice(core * BL, (core + 1) * BL)
        Xp = np.zeros((BL, NI, NST, CP), f32)
        Xp[..., :C] = X[sl]
        xr = Xp[:, :, :, cmap]                         # (BL, NI, 6, 128, 17)
        xr = xr.transpose(1, 2, 3, 4, 0).reshape(NI * NST, 128, ZF)
        with np.errstate(divide="ignore"):
            xr = np.clip(1.0 / xr, -5.9e4, 5.9e4)      # signed, inf-free recip
        xsA = rowsumW0[None, :, None] * xsA_s[sl].transpose(1, 2, 0).reshape(
            NI * NST, 1, BL)                           # (NI*6, 128, BL)
        tiles = np.zeros((NEV + NST, 128, XF), np.float16)  # +NST zero-pad
        tiles[1:NEV, :, :ZF] = xr
        tiles[1:NEV, :, ZF:] = xsA
        X0p = np.zeros((BL, CP), f32)
        X0p[:, :C] = X0[sl]
        xr0t = X0p[:, cmap].transpose(1, 2, 0).reshape(128, ZF)
        with np.errstate(divide="ignore"):
            tiles[0, :, :ZF] = np.clip(1.0 / xr0t, -5.9e4, 5.9e4)
        xr_cores.append(np.ascontiguousarray(tiles))

    # M0 stationaries (lhsT layout): M0a_{jm} = (-2 a_jm) * W0F, W0F[o,p]=W0[o,p%64]
    W0F = np.concatenate([fW0, fW0], axis=1).astype(f32)      # (128, 128)
    mats = [np.ascontiguousarray((np.float32(-2.0 * _A[j, m]) * W0F).T)
            for (j, m) in _JM]
    mats.append(np.eye(128, dtype=f32))                       # identity last
    M0cat = np.concatenate(mats, axis=1).astype(np.float16)   # (128, (NM+1)*128)

    B0 = np.stack([np.ascontiguousarray(fW0 @ y0[c * BL:(c + 1) * BL].T)
                   for c in range(N_CORES)]).astype(np.float16)

    return xr_cores, M0cat, B0, y0, S_all, S0


def _host_reconstruct(rq_cores, y0, S_all, S0, lW, lb):
    f32 = np.float32
    # stack cores on batch axis: rq_full [NEV, 128, B]
    rq = np.concatenate([rq_cores[c] for c in range(N_CORES)], axis=2).astype(f32)
    K = -2.0 * (rq[:, :HID, :] + rq[:, HID:, :]).transpose(0, 2, 1)  # (NEV, B, HID)
    # add S_m per eval
    K[0] += S0[:, None]
    K[1:] += S_all.reshape(B, NI * NST).T[:, :, None]
    bvec = _A[7]
    ysol = np.zeros((B, T, HID), f32)
    ysol[:, 0] = y0
    y = y0.copy()
    for i in range(NI):
        k1 = K[0] if i == 0 else K[1 + (i - 1) * NST + 5]
        dy = np.float32(bvec[1]) * k1
        for m in range(3, 8):        # b2 == 0
            dy += np.float32(bvec[m]) * K[1 + i * NST + (m - 2)]
        y = y + dy
        ysol[:, i + 1] = y
    return ysol


# ------------------------------------------------------------- device kernel

def _patch_act_tables():
    """Keep Exp/Ln only in their shared table set so a single
    ACT_TABLE_LOAD is hoisted (see baseline)."""
    import concourse.bacc as bacc
    import concourse.hw_specs as hw_specs
    import concourse.mybir as mybir

    if getattr(bacc, "_act_tables_patched", False):
        return
    Tt = mybir.ActivationFunctionType
    orig = hw_specs.get_activation_tables

    def patched(arch):
        tabs = orig(arch)
        for name, s in tabs.items():
            if name != "natural_log_exp_and_others":
                s.discard(Tt.Exp)
                s.discard(Tt.Ln)
        return tabs

    bacc.get_activation_tables = patched
    bacc._act_tables_patched = True


def _build(use_frep=False):
    import concourse.bass as bass
    import concourse.bacc as bacc
    import concourse.mybir as mybir
    import concourse.tile as tile

    _patch_act_tables()
    AF = mybir.ActivationFunctionType
    ALU = mybir.AluOpType
    f32 = mybir.dt.float32
    f16 = mybir.dt.float16

    nc = bacc.Bacc("TRN2", num_devices=N_CORES)

    d_xr = nc.dram_tensor("xr", [NEV + NST, 128, XF], f16, kind="ExternalInput")
    d_M0 = nc.dram_tensor("M0cat", [128, (NM + 1) * 128], f16, kind="ExternalInput")
    d_B0 = nc.dram_tensor("B0", [128, BL], f16, kind="ExternalInput")
    d_fWhT = nc.dram_tensor("fWhT", [WID, 3 * WID], f16, kind="ExternalInput")
    d_fWoT = nc.dram_tensor("fWoT", [WID, NCHUNK * 128], f16, kind="ExternalInput")
    d_b0 = nc.dram_tensor("fb0c", [WID, 1], f32, kind="ExternalInput")
    d_bh = nc.dram_tensor("fbhc", [WID, 3], f32, kind="ExternalInput")
    d_Frep = nc.dram_tensor("Frep", [128, ZF], f32, kind="ExternalInput")
    d_rq = nc.dram_tensor("rq", [NEV, 128, BL], f16, kind="ExternalOutput")

    m0_col = {jm: 128 * k for k, jm in enumerate(_JM)}
    id_col = 128 * NM

    with tile.TileContext(nc) as tc, \
         nc.allow_low_precision("fp16 rq stream validated offline (4.7e-3)"):
        with tc.tile_pool(name="const", bufs=1) as cst, \
             tc.tile_pool(name="xr", bufs=1) as xrp, \
             tc.tile_pool(name="h", bufs=2) as hp, \
             tc.tile_pool(name="big", bufs=2) as bigp, \
             tc.tile_pool(name="rqs", bufs=1) as rqp, \
             tc.tile_pool(name="dd", bufs=2) as ddp, \
             tc.tile_pool(name="u1", bufs=2, space="PSUM") as u1p, \
             tc.tile_pool(name="lay", bufs=2, space="PSUM") as layp, \
             tc.tile_pool(name="ep", bufs=2, space="PSUM") as epp, \
             tc.tile_pool(name="z", bufs=2, space="PSUM") as zap:

            M0_s = cst.tile([128, (NM + 1) * 128], f16)
            fWhT_s = cst.tile([WID, 3 * WID], f16)
            fWoT_s = cst.tile([WID, NCHUNK * 128], f16)
            b0_s = cst.tile([WID, 1], f32)
            bh_s = cst.tile([WID, 3], f32)
            B_s = cst.tile([128, BL], f16)       # base carry W0 @ y_n
            D2_s = cst.tile([128, BL], f16)
            Frep_s = cst.tile([128, ZF], f32)

            nc.sync.dma_start(M0_s[:, :], d_M0.ap()[:, :])
            nc.sync.dma_start(fWhT_s[:, :], d_fWhT.ap()[:, :])
            nc.sync.dma_start(fWoT_s[:, :], d_fWoT.ap()[:, :])
            nc.sync.dma_start(b0_s[:, :], d_b0.ap()[:, :])
            nc.sync.dma_start(bh_s[:, :], d_bh.ap()[:, :])
            nc.sync.dma_start(B_s[:, :], d_B0.ap()[:, :])
            if use_frep:
                nc.sync.dma_start(Frep_s[:, :], d_Frep.ap()[:, :])

            warm = cst.tile([1, 1], f32)
            nc.scalar.activation(warm[:, :], b0_s[0:1, 0:1], AF.Exp)
            nc.scalar.activation(warm[:, :], warm[:, :], AF.Ln, bias=1.0)

            xr_flat = d_xr.ap()

            # per-slot merged rq tiles (fp16)
            mrg = [rqp.tile([128, BL], f16, tag=f"mrg{s}", name=f"mrg{s}")
                   for s in range(NST)]
            Dt = [rqp.tile([128, BL], f16, tag=f"D{s}", name=f"D{s}")
                  for s in range(NST - 1)]   # D for stages 3..7
            slotread = [xrp.tile([128, XF], f16, tag=f"xrs{s}", name=f"xrs{s}")
                        for s in range(NST)]

            def eval_chain(u1ps, xrt, merged):
                """u1ps: assembled PSUM [128, BL]. Emits MLP + head; writes
                the head reduction into merged [128, BL] fp16."""
                e0 = epp.tile([WID, BL], f32, tag="e")
                nc.scalar.activation(e0[:, :], u1ps[:, :], AF.Exp, bias=b0_s[:, 0:1])
                h = hp.tile([WID, BL], f16, tag="h", bufs=3)
                nc.scalar.activation(h[:, :], e0[:, :], AF.Ln, bias=1.0)
                for l in range(3):
                    pl = layp.tile([WID, BL], f32, tag="lay")
                    nc.tensor.matmul(pl[:, :], fWhT_s[:, 128 * l:128 * (l + 1)],
                                     h[:, :], start=True, stop=True)
                    el = epp.tile([WID, BL], f32, tag="e")
                    nc.scalar.activation(el[:, :], pl[:, :], AF.Exp,
                                         bias=bh_s[:, l:l + 1])
                    h = hp.tile([WID, BL], f16, tag="h", bufs=3)
                    nc.scalar.activation(h[:, :], el[:, :], AF.Ln, bias=1.0)

                zps = zap.tile([128, ZF], f32, tag="z")
                for q in range(NCHUNK):
                    nc.tensor.matmul(zps[:, 8 * q:8 * (q + 1)],
                                     fWoT_s[:, 128 * q:128 * (q + 1)],
                                     h[:, :], start=True, stop=True,
                                     skip_group_check=True)

                E = bigp.tile([128, ZF], f32, tag="E")
                nc.scalar.activation(E[:, 0:ZA], zps[:, 0:ZA], AF.Exp, scale=2.0)
                nc.scalar.activation(E[:, ZA:ZF], zps[:, ZA:ZF], AF.Exp, scale=2.0)
                if use_frep:
                    nc.vector.tensor_tensor(E[:, :], E[:, :], Frep_s[:, :],
                                            op=ALU.mult)
                # qd = 1/((E+1) * xrinv)  (xr tile holds clamped 1/xrep)
                pp = ddp.tile([128, ZF], f32, tag="pp")
                nc.vector.scalar_tensor_tensor(pp[:, :], E[:, :], 1.0,
                                               xrt[:, 0:ZF],
                                               op0=ALU.add, op1=ALU.mult)
                qd = bigp.tile([128, ZF], f32, tag="qd")
                nc.vector.reciprocal_approx_fast(qd[:, :], pp[:, :])
                nc.vector.tensor_reduce(
                    merged[:, :],
                    qd[:, :].rearrange("p (q b) -> p b q", q=NCHUNK),
                    axis=mybir.AxisListType.X, op=ALU.add)


            def u1_batch(u1, j, Dj):
                """Off-chain part of u1_j assembly: id-MM + old-rq terms."""
                nc.tensor.matmul(u1[:, :], M0_s[:, id_col:id_col + 128],
                                 Dj[:, :], start=True, stop=False,
                                 skip_group_check=True)
                for m in range(1, j - 1):
                    if _A[j, m] == 0.0:
                        continue
                    col = m0_col[(j, m)]
                    rhs = mrg[NST - 1] if m == 1 else mrg[m - 2]
                    nc.tensor.matmul(u1[:, :], M0_s[:, col:col + 128],
                                     rhs[:, :], start=False, stop=False,
                                     skip_group_check=True)

            def u1_chain(u1, j, prev):
                """Chain head: last M0a term consuming the previous rq."""
                col = m0_col[(j, j - 1)]
                nc.tensor.matmul(u1[:, :], M0_s[:, col:col + 128],
                                 prev[:, :], start=False, stop=True,
                                 skip_group_check=True)

            # ---------------- pre-loop ----------------
            xr0 = xrp.tile([128, XF], f16, tag="xr0")
            nc.sync.dma_start(xr0[:, :], xr_flat[bass.DynSlice(0, 1), :, :])
            u1i = u1p.tile([128, BL], f32, tag="u1")
            nc.tensor.matmul(u1i[:, :], M0_s[:, id_col:id_col + 128], B_s[:, :],
                             start=True, stop=True, skip_group_check=True)
            # initial eval writes the slot-5 tile (rq_1 for interval 0)
            eval_chain(u1i, xr0, mrg[NST - 1])
            nc.sync.dma_start(d_rq.ap()[bass.DynSlice(0, 1), :, :],
                              mrg[NST - 1][:, :])

            nc.sync.dma_start(slotread[0][:, :], xr_flat[bass.DynSlice(1, 1), :, :])
            nc.sync.dma_start(slotread[1][:, :], xr_flat[bass.DynSlice(2, 1), :, :])

            # D_2 and u1_2 batch for interval 0
            nc.vector.tensor_tensor(D2_s[:, :], B_s[:, :], slotread[0][:, ZF:XF],
                                    op=ALU.add)
            u1_first = u1p.tile([128, BL], f32, tag="u1")
            u1_batch(u1_first, 2, D2_s)

            u1_tiles = {2: u1_first}

            hints = (mybir.EngineType.PE, mybir.EngineType.Activation,
                     mybir.EngineType.DVE, mybir.EngineType.SP)
            with tc.For_i(0, NI, 1, hint_engines=hints,
                          staggered_reset=STAGGERED) as iv:
                for s in range(NST):
                    j = s + 2

                    # ---- chain: finish u1_j and run the eval
                    u1 = u1_tiles.pop(j)
                    prev = mrg[NST - 1] if s == 0 else mrg[s - 1]
                    u1_chain(u1, j, prev)
                    eval_chain(u1, slotread[s], mrg[s])

                    nc.sync.dma_start(
                        d_rq.ap()[bass.DynSlice(iv * NST + (s + 1), 1), :, :],
                        mrg[s][:, :])

                    nc.sync.dma_start(
                        slotread[(s + 2) % NST][:, :],
                        xr_flat[bass.DynSlice(iv * NST + (s + 3), 1), :, :])

                    # ---- off-chain: D + u1 batch for stage j+1 (or next
                    # interval's stage 2 at s == 5)
                    if s < NST - 1:
                        nc.vector.tensor_tensor(Dt[s][:, :], B_s[:, :],
                                                slotread[s + 1][:, ZF:XF],
                                                op=ALU.add)
                        u1n = u1p.tile([128, BL], f32, tag="u1", name=f"u1n{s}")
                        u1_batch(u1n, j + 1, Dt[s])
                        u1_tiles[j + 1] = u1n
                    else:
                        # B_{n+1} = u1_7 content (stage-7 state = y_{n+1})
                        nc.vector.tensor_copy(B_s[:, :], u1[:, :])
                        nc.vector.tensor_tensor(D2_s[:, :], B_s[:, :],
                                                slotread[0][:, ZF:XF],
                                                op=ALU.add)
                        u1n = u1p.tile([128, BL], f32, tag="u1", name="u1n5")
                        u1_batch(u1n, 2, D2_s)
                        u1_tiles[2] = u1n

    nc.compile()
    return nc


# ----------------------------------------------------------------- interface

def kernel(ts, ys, iW0, ib0, iWh, ibh, iWo, ibo, fW0, fb0, fWh, fbh, fWo, fbo,
           lW, lb):
    from concourse import bass_utils

    f32 = np.float32
    to_np = lambda a: np.asarray(a, dtype=f32)
    ts, ys = to_np(ts), to_np(ys)
    iW0, ib0, iWh, ibh = to_np(iW0), to_np(ib0), to_np(iWh), to_np(ibh)
    iWo, ibo = to_np(iWo), to_np(ibo)
    fW0, fb0, fWh, fbh = to_np(fW0), to_np(fb0), to_np(fWh), to_np(fbh)
    fWo, fbo, lW, lb = to_np(fWo), to_np(fbo), to_np(lW), to_np(lb)

    xr_cores, M0cat, B0, y0, S_all, S0 = _host_precompute(
        ts, ys, iW0, ib0, iWh, ibh, iWo, ibo, fW0, fWo)

    # c-major permuted fWo (baseline layout) + Frep fallback
    perm = np.zeros(CP * HID, np.int64) - 1
    csrc = np.arange(C)
    for h_i in range(HID):
        perm[csrc * HID + h_i] = h_i * C + csrc
    fWo_cm = np.zeros((CP * HID, WID), f32)
    fbo_cm = np.zeros((CP * HID,), f32)
    valid = perm >= 0
    fWo_cm[valid] = fWo[perm[valid]]
    fbo_cm[valid] = fbo[perm[valid]]
    fWoT = np.ascontiguousarray(
        np.concatenate([fWo_cm[128 * q:128 * (q + 1)].T for q in range(NCHUNK)],
                       axis=1)).astype(np.float16)
    Frep = np.exp(2.0 * fbo_cm.reshape(NCHUNK, 128)).T
    Frep = np.repeat(Frep[:, :, None], 8, axis=2).reshape(128, ZF).astype(f32)

    use_frep = bool(np.any(fbo))
    global _COMPILED
    if _COMPILED is None or _COMPILED[0] != use_frep:
        _COMPILED = (use_frep, _build(use_frep=use_frep))
    nc = _COMPILED[1]

    fWhT = np.ascontiguousarray(
        np.concatenate([fWh[k].T for k in range(3)], axis=1)).astype(np.float16)

    in_maps = []
    for core in range(N_CORES):
        in_maps.append({
            "xr": xr_cores[core],
            "M0cat": M0cat,
            "B0": B0[core],
            "fWhT": fWhT,
            "fWoT": fWoT,
            "fb0c": fb0[:, None],
            "fbhc": np.ascontiguousarray(fbh.T),
            "Frep": Frep,
        })

    global _LAST_IN_MAPS
    _LAST_IN_MAPS = in_maps
    res = bass_utils.run_bass_kernel_spmd(nc, in_maps, core_ids=list(range(N_CORES)))

    rq_cores = [res.results[core]["rq"] for core in range(N_CORES)]
    ysol = _host_reconstruct(rq_cores, y0, S_all, S0, lW, lb)
    out = ysol @ lW.T + lb[None, None, :]
    return out.astype(f32)


if __name__ == "__main__":
    pass


# revision 9
# speedup vs baseline: 3.1365x; 1.0534x over previous
"""NeuralCDE forward on 8 Trainium2 NeuronCores — v2.

The reference integrates with RK4 x 4 substeps (16 MLP evals/interval).
The wall-clock is bound by the *serial* eval chain (batch width is nearly
free), so v2:

1. Integrates with DOPRI5 + FSAL: 6 evals/interval (vs 16), validated
   rel_err ~1.8e-3 vs the reference (gate 2e-2).
2. Shortens each eval's chain by linearity-folding the stage combines:
   the stage state y_j is never materialized on-chain. The first-layer
   pre-activation u1_j = W0 @ y_j decomposes as
     u1_j = B_n + xsA_j + sum_m (-2 a_jm) * (W0.fold) @ rq_m
   where rq_m is the per-eval head reduction, B_n = W0 @ y_n carries via
   B_{n+1} = u1_7 (dopri5's 7th stage state IS y_{n+1}), and xsA_j is a
   host-precomputed rank-1 term. All combines are PE matmuls with
   pre-scaled stationaries (M0a); the old S-matmul/DVE stage tail is gone.
3. Streams rq out; the host reconstructs y (K_m = S_m - 2*fold(rq_m)) and
   applies the readout. fbo == 0 assumed (checked; Frep fallback built
   on demand as in the baseline).
4. Head is split into two chunk groups (9+8) so Exp/DVE overlap the
   chunk matmuls; softplus stays Exp+Ln on ACT (single act-table set).
"""

import numpy as np

N_CORES = 8
T = 128
B = 64
OBS = 32
HID = 64
WID = 128
OUT = 32
C = OBS + 1          # 33
CP = 34              # padded C (even)
NCHUNK = 17          # 2176 / 128
ZF = NCHUNK * 8      # 136
XF = ZF + 8          # 144: xrep ++ xsA
NI = T - 1           # 127 intervals
NST = 6              # dopri5 evals per interval (stages 2..7)
NEV = 1 + NI * NST   # total evals incl. initial k1
BL = B // N_CORES    # 8 per core
NQA = 9              # chunks in head group A
NQB = NCHUNK - NQA   # 8
ZA = NQA * 8         # 72
STAGGERED = True

_COMPILED = None
_LAST_IN_MAPS = None

# dopri5 tableau (row 7 = b; FSAL)
_A = np.zeros((8, 8))
_A[2, 1] = 1 / 5
_A[3, 1:3] = [3 / 40, 9 / 40]
_A[4, 1:4] = [44 / 45, -56 / 15, 32 / 9]
_A[5, 1:5] = [19372 / 6561, -25360 / 2187, 64448 / 6561, -212 / 729]
_A[6, 1:6] = [9017 / 3168, -355 / 33, 46732 / 5247, 49 / 176, -5103 / 18656]
_A[7, 1:7] = [35 / 384, 0.0, 500 / 1113, 125 / 192, -2187 / 6784, 11 / 84]
_CS = [0.0, 0.0, 1 / 5, 3 / 10, 4 / 5, 8 / 9, 1.0, 1.0]
# (j, m) pairs with a_jm != 0, in emission order per stage
_JM = [(j, m) for j in range(2, 8) for m in range(1, j) if _A[j, m] != 0.0]
NM = len(_JM)        # 20


# ----------------------------------------------------------------- host math

def _host_precompute(ts, ys, iW0, ib0, iWh, ibh, iWo, ibo, fW0, fWo):
    f32 = np.float32
    ts = ts.astype(f32)
    ys = ys.astype(f32)

    tys = np.concatenate([np.broadcast_to(ts[None, :, None], (B, T, 1)), ys], axis=-1)
    dts = ts[1:] - ts[:-1]
    diffs = (tys[:, 1:] - tys[:, :-1]) / dts[None, :, None]
    deriv = np.concatenate([diffs[:, :1], diffs], axis=1)
    d0 = deriv[:, :-1]
    d1 = deriv[:, 1:]
    cc = (3.0 * diffs - 2.0 * d0 - d1) / dts[None, :, None]
    bb = (d0 + d1 - 2.0 * diffs) / (dts * dts)[None, :, None]

    # X[b, i, jj, c] = h * xdot at stage (jj+2)'s c-point; X0 = initial c=0
    cpts = np.array([_CS[j] for j in range(2, 8)], f32)
    s = (cpts[None, :] * dts[:, None])[None, :, :, None]
    X = (d0[:, :, None, :] + 2.0 * cc[:, :, None, :] * s
         + 3.0 * bb[:, :, None, :] * s * s) * dts[None, :, None, None]
    X = X.astype(f32)                                  # (B, NI, 6, C)
    X0 = (d0[:, 0] * dts[0]).astype(f32)               # (B, C)

    S_all = X.sum(-1)                                  # (B, NI, 6)
    S0 = X0.sum(-1)                                    # (B,)
    rowsumW0 = fW0.sum(axis=1).astype(f32)             # (128,)

    # y0 via init MLP
    relu = lambda v: np.maximum(v, 0.0, dtype=f32)
    h = relu(tys[:, 0] @ iW0.T + ib0[None, :])
    for k in range(iWh.shape[0]):
        h = relu(h @ iWh[k].T + ibh[k][None, :])
    y0 = (h @ iWo.T + ibo[None, :]).astype(f32)        # (B, HID)

    # xsA scalars per (i, jj): sum_{m<j} a_jm * S_m  -> (B, NI, 6)
    xsA_s = np.zeros((B, NI, NST), f32)
    for jj in range(NST):
        j = jj + 2
        for m in range(1, j):
            a = _A[j, m]
            if a == 0.0:
                continue
            if m == 1:
                Sm = np.concatenate([S0[:, None], S_all[:, :-1, 5]], axis=1)  # (B, NI)
            else:
                Sm = S_all[:, :, m - 2]
            xsA_s[:, :, jj] += np.float32(a) * Sm

    # per-core xr tiles: [NEV(+pad), 128, XF]: xrep cols 0:136, xsA cols 136:144
    q_idx = np.arange(NCHUNK)
    part_half = np.arange(128) // 64
    cmap = (2 * q_idx[None, :] + part_half[:, None])   # (128, 17)

    xr_cores = []
    for core in range(N_CORES):
        sl = slice(core * BL, (core + 1) * BL)
        Xp = np.zeros((BL, NI, NST, CP), f32)
        Xp[..., :C] = X[sl]
        xr = Xp[:, :, :, cmap]                         # (BL, NI, 6, 128, 17)
        xr = xr.transpose(1, 2, 3, 4, 0).reshape(NI * NST, 128, ZF)
        with np.errstate(divide="ignore"):
            xr = np.clip(1.0 / xr, -5.9e4, 5.9e4)      # signed, inf-free recip
        xsA = rowsumW0[None, :, None] * xsA_s[sl].transpose(1, 2, 0).reshape(
            NI * NST, 1, BL)                           # (NI*6, 128, BL)
        tiles = np.zeros((NEV + NST, 128, XF), np.float16)  # +NST zero-pad
        tiles[1:NEV, :, :ZF] = xr
        tiles[1:NEV, :, ZF:] = xsA
        X0p = np.zeros((BL, CP), f32)
        X0p[:, :C] = X0[sl]
        xr0t = X0p[:, cmap].transpose(1, 2, 0).reshape(128, ZF)
        with np.errstate(divide="ignore"):
            tiles[0, :, :ZF] = np.clip(1.0 / xr0t, -5.9e4, 5.9e4)
        xr_cores.append(np.ascontiguousarray(tiles))

    # M0 stationaries (lhsT layout): M0a_{jm} = (-2 a_jm) * W0F, W0F[o,p]=W0[o,p%64]
    W0F = np.concatenate([fW0, fW0], axis=1).astype(f32)      # (128, 128)
    mats = [np.ascontiguousarray((np.float32(-2.0 * _A[j, m]) * W0F).T)
            for (j, m) in _JM]
    mats.append(np.eye(128, dtype=f32))                       # identity last
    M0cat = np.concatenate(mats, axis=1).astype(np.float16)   # (128, (NM+1)*128)

    B0 = np.stack([np.ascontiguousarray(fW0 @ y0[c * BL:(c + 1) * BL].T)
                   for c in range(N_CORES)]).astype(np.float16)

    return xr_cores, M0cat, B0, y0, S_all, S0


def _host_reconstruct(rq_cores, y0, S_all, S0, lW, lb):
    f32 = np.float32
    # stack cores on batch axis: rq_full [NEV, 128, B]
    rq = np.concatenate([rq_cores[c] for c in range(N_CORES)], axis=2).astype(f32)
    K = -2.0 * (rq[:, :HID, :] + rq[:, HID:, :]).transpose(0, 2, 1)  # (NEV, B, HID)
    # add S_m per eval
    K[0] += S0[:, None]
    K[1:] += S_all.reshape(B, NI * NST).T[:, :, None]
    bvec = _A[7]
    ysol = np.zeros((B, T, HID), f32)
    ysol[:, 0] = y0
    y = y0.copy()
    for i in range(NI):
        k1 = K[0] if i == 0 else K[1 + (i - 1) * NST + 5]
        dy = np.float32(bvec[1]) * k1
        for m in range(3, 8):        # b2 == 0
            dy += np.float32(bvec[m]) * K[1 + i * NST + (m - 2)]
        y = y + dy
        ysol[:, i + 1] = y
    return ysol


# ------------------------------------------------------------- device kernel

def _patch_act_tables():
    """Keep Exp/Ln only in their shared table set so a single
    ACT_TABLE_LOAD is hoisted (see baseline)."""
    import concourse.bacc as bacc
    import concourse.hw_specs as hw_specs
    import concourse.mybir as mybir

    if getattr(bacc, "_act_tables_patched", False):
        return
    Tt = mybir.ActivationFunctionType
    orig = hw_specs.get_activation_tables

    def patched(arch):
        tabs = orig(arch)
        for name, s in tabs.items():
            if name != "natural_log_exp_and_others":
                s.discard(Tt.Exp)
                s.discard(Tt.Ln)
        return tabs

    bacc.get_activation_tables = patched
    bacc._act_tables_patched = True


def _build(use_frep=False):
    import concourse.bass as bass
    import concourse.bacc as bacc
    import concourse.mybir as mybir
    import concourse.tile as tile

    _patch_act_tables()
    AF = mybir.ActivationFunctionType
    ALU = mybir.AluOpType
    f32 = mybir.dt.float32
    f16 = mybir.dt.float16

    nc = bacc.Bacc("TRN2", num_devices=N_CORES)

    d_xr = nc.dram_tensor("xr", [NEV + NST, 128, XF], f16, kind="ExternalInput")
    d_M0 = nc.dram_tensor("M0cat", [128, (NM + 1) * 128], f16, kind="ExternalInput")
    d_B0 = nc.dram_tensor("B0", [128, BL], f16, kind="ExternalInput")
    d_fWhT = nc.dram_tensor("fWhT", [WID, 3 * WID], f16, kind="ExternalInput")
    d_fWoT = nc.dram_tensor("fWoT", [WID, NCHUNK * 128], f16, kind="ExternalInput")
    d_b0 = nc.dram_tensor("fb0c", [WID, 1], f32, kind="ExternalInput")
    d_bh = nc.dram_tensor("fbhc", [WID, 3], f32, kind="ExternalInput")
    d_Frep = nc.dram_tensor("Frep", [128, ZF], f32, kind="ExternalInput")
    d_rq = nc.dram_tensor("rq", [NEV, 128, BL], f16, kind="ExternalOutput")

    m0_col = {jm: 128 * k for k, jm in enumerate(_JM)}
    id_col = 128 * NM

    with tile.TileContext(nc) as tc, \
         nc.allow_low_precision("fp16 rq stream validated offline (4.7e-3)"):
        with tc.tile_pool(name="const", bufs=1) as cst, \
             tc.tile_pool(name="xr", bufs=1) as xrp, \
             tc.tile_pool(name="h", bufs=2) as hp, \
             tc.tile_pool(name="big", bufs=2) as bigp, \
             tc.tile_pool(name="rqs", bufs=1) as rqp, \
             tc.tile_pool(name="dd", bufs=2) as ddp, \
             tc.tile_pool(name="u1", bufs=2, space="PSUM") as u1p, \
             tc.tile_pool(name="lay", bufs=2, space="PSUM") as layp, \
             tc.tile_pool(name="ep", bufs=2, space="PSUM") as epp, \
             tc.tile_pool(name="z", bufs=2, space="PSUM") as zap:

            M0_s = cst.tile([128, (NM + 1) * 128], f16)
            fWhT_s = cst.tile([WID, 3 * WID], f16)
            fWoT_s = cst.tile([WID, NCHUNK * 128], f16)
            b0_s = cst.tile([WID, 1], f32)
            bh_s = cst.tile([WID, 3], f32)
            B_s = cst.tile([128, BL], f16)       # base carry W0 @ y_n
            D2_s = cst.tile([128, BL], f16)
            Frep_s = cst.tile([128, ZF], f32)

            nc.sync.dma_start(M0_s[:, :], d_M0.ap()[:, :])
            nc.sync.dma_start(fWhT_s[:, :], d_fWhT.ap()[:, :])
            nc.sync.dma_start(fWoT_s[:, :], d_fWoT.ap()[:, :])
            nc.sync.dma_start(b0_s[:, :], d_b0.ap()[:, :])
            nc.sync.dma_start(bh_s[:, :], d_bh.ap()[:, :])
            nc.sync.dma_start(B_s[:, :], d_B0.ap()[:, :])
            if use_frep:
                nc.sync.dma_start(Frep_s[:, :], d_Frep.ap()[:, :])

            warm = cst.tile([1, 1], f32)
            nc.scalar.activation(warm[:, :], b0_s[0:1, 0:1], AF.Exp)
            nc.scalar.activation(warm[:, :], warm[:, :], AF.Ln, bias=1.0)

            xr_flat = d_xr.ap()

            # per-slot merged rq tiles (fp16)
            mrg = [rqp.tile([128, BL], f16, tag=f"mrg{s}", name=f"mrg{s}")
                   for s in range(NST)]
            Dt = [rqp.tile([128, BL], f16, tag=f"D{s}", name=f"D{s}")
                  for s in range(NST - 1)]   # D for stages 3..7
            slotread = [xrp.tile([128, XF], f16, tag=f"xrs{s}", name=f"xrs{s}")
                        for s in range(NST)]

            def eval_chain(u1ps, xrt, merged):
                """u1ps: assembled PSUM [128, BL]. Emits MLP + head; writes
                the head reduction into merged [128, BL] fp16."""
                e0 = epp.tile([WID, BL], f32, tag="e")
                nc.scalar.activation(e0[:, :], u1ps[:, :], AF.Exp, bias=b0_s[:, 0:1])
                h = hp.tile([WID, BL], f16, tag="h", bufs=3)
                nc.scalar.activation(h[:, :], e0[:, :], AF.Ln, bias=1.0)
                for l in range(3):
                    pl = layp.tile([WID, BL], f32, tag="lay")
                    nc.tensor.matmul(pl[:, :], fWhT_s[:, 128 * l:128 * (l + 1)],
                                     h[:, :], start=True, stop=True)
                    el = epp.tile([WID, BL], f32, tag="e")
                    nc.scalar.activation(el[:, :], pl[:, :], AF.Exp,
                                         bias=bh_s[:, l:l + 1])
                    h = hp.tile([WID, BL], f16, tag="h", bufs=3)
                    nc.scalar.activation(h[:, :], el[:, :], AF.Ln, bias=1.0)

                zps = zap.tile([128, ZF], f32, tag="z")
                for q in range(NCHUNK):
                    nc.tensor.matmul(zps[:, 8 * q:8 * (q + 1)],
                                     fWoT_s[:, 128 * q:128 * (q + 1)],
                                     h[:, :], start=True, stop=True,
                                     skip_group_check=True)

                E = bigp.tile([128, ZF], f32, tag="E")
                nc.scalar.activation(E[:, :], zps[:, :], AF.Exp, scale=2.0)
                if use_frep:
                    nc.vector.tensor_tensor(E[:, :], E[:, :], Frep_s[:, :],
                                            op=ALU.mult)
                # qd = 1/((E+1) * xrinv)  (xr tile holds clamped 1/xrep)
                pp = ddp.tile([128, ZF], f32, tag="pp")
                nc.vector.scalar_tensor_tensor(pp[:, :], E[:, :], 1.0,
                                               xrt[:, 0:ZF],
                                               op0=ALU.add, op1=ALU.mult)
                qd = bigp.tile([128, ZF], f32, tag="qd")
                nc.vector.reciprocal_approx_fast(qd[:, :], pp[:, :])
                nc.vector.tensor_reduce(
                    merged[:, :],
                    qd[:, :].rearrange("p (q b) -> p b q", q=NCHUNK),
                    axis=mybir.AxisListType.X, op=ALU.add)


            def u1_batch(u1, j, Dj):
                """Off-chain part of u1_j assembly: id-MM + old-rq terms."""
                nc.tensor.matmul(u1[:, :], M0_s[:, id_col:id_col + 128],
                                 Dj[:, :], start=True, stop=False,
                                 skip_group_check=True)
                for m in range(1, j - 1):
                    if _A[j, m] == 0.0:
                        continue
                    col = m0_col[(j, m)]
                    rhs = mrg[NST - 1] if m == 1 else mrg[m - 2]
                    nc.tensor.matmul(u1[:, :], M0_s[:, col:col + 128],
                                     rhs[:, :], start=False, stop=False,
                                     skip_group_check=True)

            def u1_chain(u1, j, prev):
                """Chain head: last M0a term consuming the previous rq."""
                col = m0_col[(j, j - 1)]
                nc.tensor.matmul(u1[:, :], M0_s[:, col:col + 128],
                                 prev[:, :], start=False, stop=True,
                                 skip_group_check=True)

            # ---------------- pre-loop ----------------
            xr0 = xrp.tile([128, XF], f16, tag="xr0")
            nc.sync.dma_start(xr0[:, :], xr_flat[bass.DynSlice(0, 1), :, :])
            u1i = u1p.tile([128, BL], f32, tag="u1")
            nc.tensor.matmul(u1i[:, :], M0_s[:, id_col:id_col + 128], B_s[:, :],
                             start=True, stop=True, skip_group_check=True)
            # initial eval writes the slot-5 tile (rq_1 for interval 0)
            eval_chain(u1i, xr0, mrg[NST - 1])
            nc.sync.dma_start(d_rq.ap()[bass.DynSlice(0, 1), :, :],
                              mrg[NST - 1][:, :])

            nc.sync.dma_start(slotread[0][:, :], xr_flat[bass.DynSlice(1, 1), :, :])
            nc.sync.dma_start(slotread[1][:, :], xr_flat[bass.DynSlice(2, 1), :, :])

            # D_2 and u1_2 batch for interval 0
            nc.vector.tensor_tensor(D2_s[:, :], B_s[:, :], slotread[0][:, ZF:XF],
                                    op=ALU.add)
            u1_first = u1p.tile([128, BL], f32, tag="u1")
            u1_batch(u1_first, 2, D2_s)

            u1_tiles = {2: u1_first}

            hints = (mybir.EngineType.PE, mybir.EngineType.Activation,
                     mybir.EngineType.DVE, mybir.EngineType.SP)
            with tc.For_i(0, NI, 1, hint_engines=hints,
                          staggered_reset=STAGGERED) as iv:
                for s in range(NST):
                    j = s + 2

                    # ---- chain: finish u1_j and run the eval
                    u1 = u1_tiles.pop(j)
                    prev = mrg[NST - 1] if s == 0 else mrg[s - 1]
                    u1_chain(u1, j, prev)
                    eval_chain(u1, slotread[s], mrg[s])

                    nc.sync.dma_start(
                        d_rq.ap()[bass.DynSlice(iv * NST + (s + 1), 1), :, :],
                        mrg[s][:, :])

                    nc.sync.dma_start(
                        slotread[(s + 2) % NST][:, :],
                        xr_flat[bass.DynSlice(iv * NST + (s + 3), 1), :, :])

                    # ---- off-chain: D + u1 batch for stage j+1 (or next
                    # interval's stage 2 at s == 5)
                    if s < NST - 1:
                        nc.vector.tensor_tensor(Dt[s][:, :], B_s[:, :],
                                                slotread[s + 1][:, ZF:XF],
                                                op=ALU.add)
                        u1n = u1p.tile([128, BL], f32, tag="u1", name=f"u1n{s}")
                        u1_batch(u1n, j + 1, Dt[s])
                        u1_tiles[j + 1] = u1n
                    else:
                        # B_{n+1} = u1_7 content (stage-7 state = y_{n+1})
                        nc.vector.tensor_copy(B_s[:, :], u1[:, :])
                        nc.vector.tensor_tensor(D2_s[:, :], B_s[:, :],
                                                slotread[0][:, ZF:XF],
                                                op=ALU.add)
                        u1n = u1p.tile([128, BL], f32, tag="u1", name="u1n5")
                        u1_batch(u1n, 2, D2_s)
                        u1_tiles[2] = u1n

    nc.compile()
    return nc


# ----------------------------------------------------------------- interface

def kernel(ts, ys, iW0, ib0, iWh, ibh, iWo, ibo, fW0, fb0, fWh, fbh, fWo, fbo,
           lW, lb):
    from concourse import bass_utils

    f32 = np.float32
    to_np = lambda a: np.asarray(a, dtype=f32)
    ts, ys = to_np(ts), to_np(ys)
    iW0, ib0, iWh, ibh = to_np(iW0), to_np(ib0), to_np(iWh), to_np(ibh)
    iWo, ibo = to_np(iWo), to_np(ibo)
    fW0, fb0, fWh, fbh = to_np(fW0), to_np(fb0), to_np(fWh), to_np(fbh)
    fWo, fbo, lW, lb = to_np(fWo), to_np(fbo), to_np(lW), to_np(lb)

    xr_cores, M0cat, B0, y0, S_all, S0 = _host_precompute(
        ts, ys, iW0, ib0, iWh, ibh, iWo, ibo, fW0, fWo)

    # c-major permuted fWo (baseline layout) + Frep fallback
    perm = np.zeros(CP * HID, np.int64) - 1
    csrc = np.arange(C)
    for h_i in range(HID):
        perm[csrc * HID + h_i] = h_i * C + csrc
    fWo_cm = np.zeros((CP * HID, WID), f32)
    fbo_cm = np.zeros((CP * HID,), f32)
    valid = perm >= 0
    fWo_cm[valid] = fWo[perm[valid]]
    fbo_cm[valid] = fbo[perm[valid]]
    fWoT = np.ascontiguousarray(
        np.concatenate([fWo_cm[128 * q:128 * (q + 1)].T for q in range(NCHUNK)],
                       axis=1)).astype(np.float16)
    Frep = np.exp(2.0 * fbo_cm.reshape(NCHUNK, 128)).T
    Frep = np.repeat(Frep[:, :, None], 8, axis=2).reshape(128, ZF).astype(f32)

    use_frep = bool(np.any(fbo))
    global _COMPILED
    if _COMPILED is None or _COMPILED[0] != use_frep:
        _COMPILED = (use_frep, _build(use_frep=use_frep))
    nc = _COMPILED[1]

    fWhT = np.ascontiguousarray(
        np.concatenate([fWh[k].T for k in range(3)], axis=1)).astype(np.float16)

    in_maps = []
    for core in range(N_CORES):
        in_maps.append({
            "xr": xr_cores[core],
            "M0cat": M0cat,
            "B0": B0[core],
            "fWhT": fWhT,
            "fWoT": fWoT,
            "fb0c": fb0[:, None],
            "fbhc": np.ascontiguousarray(fbh.T),
            "Frep": Frep,
        })

    global _LAST_IN_MAPS
    _LAST_IN_MAPS = in_maps
    res = bass_utils.run_bass_kernel_spmd(nc, in_maps, core_ids=list(range(N_CORES)))

    rq_cores = [res.results[core]["rq"] for core in range(N_CORES)]
    ysol = _host_reconstruct(rq_cores, y0, S_all, S0, lW, lb)
    out = ysol @ lW.T + lb[None, None, :]
    return out.astype(f32)


if __name__ == "__main__":
    pass
